# revision 5
# baseline (speedup 1.0000x reference)
"""Differential attention kernel for Trainium2, 8-core SPMD.

Problem: B=2, S=2048, D=1024, 16 heads x 64 head-dim differential attention
(two softmaxes combined with a scalar lambda), with input/output projections.

Sharding: data-parallel over batch (2 groups of 4 cores) x tensor-parallel
over heads (4 heads per core). Each core computes q/k/v projections for its
4 heads, both attention softmaxes, and a partial output projection
(its heads' rows of Wo). Host sums the 4 partial outputs per batch, adds bo.

Design (driven by the TimelineSim cost model, where a matmul costs
out-free-size x pe_cycle and an ACT op costs free-size x act_cycle):
  - Projections produce transposed q/k ([dh 128 = q1|q2 stacked, S]) and
    v ([s, 4*64]) with K=128/M=128 matmuls (row-optimal).
  - Scores are computed transposed, sT[k, q], per 128-token k-chunk:
    two K=64 matmuls (halves on partition ranges 0:64 / 64:128).
  - exp() runs on ACT straight out of PSUM ([128, 1024] per chunk,
    double-buffered PSUM), mask folded into the per-partition bias and the
    1/sqrt(hd) scale into the activation scale.  ACT is the kernel's
    bottleneck engine (~266 us busy), so nothing else runs on ACT.
  - PV is orientation-flipped vs the baseline: out[q, d] with lhsT = et
    chunk [k 128, q 128] and rhs = v [k 128, d 64] -> 64-row matmuls at
    full K=128/M=128 (half the PE rows of the [d, q] orientation).  All
    8 accumulators (4 q-subchunks x 2 softmax halves) pack into ONE psum
    bank; softmax sums accumulate via 1-row ones-matmuls into a second
    bank.  PSUM 2KB zero-region semantics: only the first matmul touching
    a bank per round uses start=True, the other regions' first writes
    clear their pending-zero bytes (fresh write), later writes accumulate.
  - Normalization needs no partition broadcast in this orientation:
    DVE reciprocal of the sums column + per-partition tensor_scalar ops
    combine the halves with lambda folded in.
  - stg [q, d] is PE-transposed (plain bf16 matmul against an identity)
    into [d, q] with head pairs stacked on partitions, so the output
    projection contracts K=128 (half the PE rows of per-head K=64).
All matmuls run in bf16 with fp32 PSUM accumulation; output partials ship
as fp16 and are reduced across cores in fp32 on the host.

Engine budget per core (cost model): ACT 266us (256 exp ops), PE ~255us
(~608k matmul rows), DVE ~90us.  The emission order keeps ACT dense:
score chunks are the primary stream; projection groups are emitted
just-in-time ahead of their consumers; PV work for early (h, j) blocks is
deferred (et tiles are held in SBUF) until the v-projection has drained,
then drains as filler between score chunks.
"""

import sys

sys.path.insert(0, "/opt/trn_rl_repo")

from collections import deque
from contextlib import ExitStack

import ml_dtypes
import numpy as np

import concourse.bacc as bacc
import concourse.tile as tile
from concourse import mybir
from concourse.bass_utils import run_bass_kernel_spmd

B, S, D = 2, 2048, 1024
NH, HD = 16, 64
NCORES = 8
HPC = 4              # heads per core
QB = 512             # q block (free dim of score matmuls)
NJ = S // QB         # 4
KC = 128             # k chunk (partition dim of transposed scores)
NKC = S // KC        # 16
NDI = D // 128       # 8 contraction chunks for projections
NQC = QB // 128      # 4 q-subchunks per block (PV output partition tiles)

BF16 = mybir.dt.bfloat16
F32 = mybir.dt.float32
F16 = mybir.dt.float16
npbf16 = ml_dtypes.bfloat16

_BUILD_CACHE = {}
TRACE = False
LAST_RESULTS = None


def _build(lam: float, with_bias: bool = True, repeat: int = 1):
    nc = bacc.Bacc(None, target_bir_lowering=False)
    mult = mybir.AluOpType.mult
    subtract = mybir.AluOpType.subtract

    hst_d = nc.dram_tensor("hst", [D, S], BF16, kind="ExternalInput")
    wq_d = nc.dram_tensor("wq", [D, 2 * HPC * HD], BF16, kind="ExternalInput")
    wk_d = nc.dram_tensor("wk", [D, 2 * HPC * HD], BF16, kind="ExternalInput")
    wv_d = nc.dram_tensor("wv", [D, HPC * HD], BF16, kind="ExternalInput")
    wo_d = nc.dram_tensor("wo", [HPC * HD, D], BF16, kind="ExternalInput")
    bq_d = nc.dram_tensor("bq", [1, 2 * HPC * HD], BF16, kind="ExternalInput")
    bk_d = nc.dram_tensor("bk", [1, 2 * HPC * HD], BF16, kind="ExternalInput")
    bv_d = nc.dram_tensor("bv", [1, HPC * HD], BF16, kind="ExternalInput")
    mask_d = nc.dram_tensor("maskc", [KC, NKC], F32, kind="ExternalInput")
    id_d = nc.dram_tensor("ident", [128, 128], BF16, kind="ExternalInput")
    out_d = nc.dram_tensor("outT", [D, S], F16, kind="ExternalOutput")

    with tile.TileContext(nc) as tc, ExitStack() as ctx:
        const = ctx.enter_context(tc.tile_pool(name="const", bufs=1))
        wpool = ctx.enter_context(tc.tile_pool(name="wpool", bufs=1))
        hpool = ctx.enter_context(tc.tile_pool(name="hpool", bufs=1))
        qkpool = ctx.enter_context(tc.tile_pool(name="qkpool", bufs=1))
        vpool = ctx.enter_context(tc.tile_pool(name="vpool", bufs=1))
        epool = ctx.enter_context(tc.tile_pool(name="epool", bufs=24))
        spool = ctx.enter_context(tc.tile_pool(name="spool", bufs=1))
        gpool = ctx.enter_context(tc.tile_pool(name="gpool", bufs=8))
        opool = ctx.enter_context(tc.tile_pool(name="opool", bufs=4))
        ps_mm = ctx.enter_context(tc.tile_pool(name="ps_mm", bufs=2, space="PSUM"))
        ps_sc = ctx.enter_context(tc.tile_pool(name="ps_sc", bufs=2, space="PSUM"))
        ps_pv = ctx.enter_context(tc.tile_pool(name="ps_pv", bufs=1, space="PSUM"))
        ps_sm = ctx.enter_context(tc.tile_pool(name="ps_sm", bufs=1, space="PSUM"))

        # ---- input DMAs, spread over queues so the first projection groups
        # start as soon as possible.  SP gets wk (first consumer), ACT gets wq
        # (ACT is otherwise idle until the first exp), Pool gets hs j0 + the
        # bulk, DVE gets the small constants before its first eviction. ----
        wkt = wpool.tile([128, NDI, 512], BF16, tag="wk", name="wkt")
        wqt = wpool.tile([128, NDI, 512], BF16, tag="wq", name="wqt")
        wvt = wpool.tile([128, NDI, 256], BF16, tag="wv", name="wvt")
        hst = hpool.tile([128, NDI, S], BF16, tag="hs", name="hst")
        wk_r = wk_d[:].rearrange("(c p) m -> p c m", p=128)
        wq_r = wq_d[:].rearrange("(c p) m -> p c m", p=128)
        wv_r = wv_d[:].rearrange("(c p) m -> p c m", p=128)
        hs_r = hst_d[:].rearrange("(c p) s -> p c s", p=128)

        nc.sync.dma_start(out=wkt[:, 0:4, :], in_=wk_r[:, 0:4, :])
        nc.sync.dma_start(out=wkt[:, 4:8, :], in_=wk_r[:, 4:8, :])
        nc.scalar.dma_start(out=wqt[:, 0:4, :], in_=wq_r[:, 0:4, :])
        nc.scalar.dma_start(out=wqt[:, 4:8, :], in_=wq_r[:, 4:8, :])
        nc.gpsimd.dma_start(out=hst[:, 0:4, 0:QB], in_=hs_r[:, 0:4, 0:QB])
        nc.gpsimd.dma_start(out=hst[:, 4:8, 0:QB], in_=hs_r[:, 4:8, 0:QB])

        maskt = const.tile([KC, NKC], F32, tag="mask")
        nc.gpsimd.dma_start(out=maskt[:], in_=mask_d[:])
        ident = const.tile([128, 128], BF16, tag="ident")
        nc.gpsimd.dma_start(out=ident[:], in_=id_d[:])
        ones = const.tile([128, QB], BF16, tag="ones")
        nc.gpsimd.memset(ones[:], 1.0)

        # rest of hs on SP; v/o weights + biases on Pool
        for j in range(1, NJ):
            nc.sync.dma_start(
                out=hst[:, :, j * QB:(j + 1) * QB], in_=hs_r[:, :, j * QB:(j + 1) * QB]
            )
        nc.gpsimd.dma_start(out=wvt[:], in_=wv_r[:])
        wot = []
        for p in range(HPC // 2):
            t = wpool.tile([128, D], BF16, tag=f"wo{p}", name=f"wo{p}")
            nc.gpsimd.dma_start(out=t[:], in_=wo_d[p * 128:(p + 1) * 128, :])
            wot.append(t)
        if with_bias:
            bqt = const.tile([1, 2 * HPC * HD], BF16, tag="bq")
            nc.gpsimd.dma_start(out=bqt[:], in_=bq_d[:])
            bkt = const.tile([1, 2 * HPC * HD], BF16, tag="bk")
            nc.gpsimd.dma_start(out=bkt[:], in_=bk_d[:])
            bvt = const.tile([1, HPC * HD], BF16, tag="bv")
            nc.gpsimd.dma_start(out=bvt[:], in_=bv_d[:])

        # ---- persistent per-head tiles ----
        qT = [qkpool.tile([128, S], BF16, tag=f"qT{h}", name=f"qT{h}")
              for h in range(HPC)]
        kT = [qkpool.tile([128, S], BF16, tag=f"kT{h}", name=f"kT{h}")
              for h in range(HPC)]
        va = [vpool.tile([128, HPC * HD], BF16, tag=f"va{c}", name=f"va{c}")
              for c in range(NKC)]
        stgT = [spool.tile([128, S], BF16, tag=f"sT{p}", name=f"sT{p}")
                for p in range(HPC // 2)]

        kdone = [[False] * NJ for _ in range(HPC)]
        qdone = [[False] * NJ for _ in range(HPC)]

        def emit_qkproj_group(which, h, g):
            # one j-group of the q or k projection for head h: psum [128, 512]
            # (partitions = q1|q2 of the head), evicted bf16 into qT/kT.
            wt, bt, dst, done = (
                (wqt, bqt if with_bias else None, qT[h], qdone)
                if which == "q"
                else (wkt, bkt if with_bias else None, kT[h], kdone)
            )
            if done[h][g]:
                return
            done[h][g] = True
            ps = ps_mm.tile([128, 512], F32, tag="mm", name="psqk")
            for c in range(NDI):
                nc.tensor.matmul(
                    ps[:],
                    lhsT=wt[:, c, h * 128:(h + 1) * 128],
                    rhs=hst[:, c, g * QB:(g + 1) * QB],
                    start=(c == 0),
                    stop=(with_bias is False and c == NDI - 1),
                )
            if with_bias:
                nc.tensor.matmul(
                    ps[:],
                    lhsT=bt[0:1, h * 128:(h + 1) * 128],
                    rhs=ones[0:1, :],
                    start=False,
                    stop=True,
                )
            nc.vector.tensor_copy(dst[:, g * QB:(g + 1) * QB], ps[:])

        def emit_vproj_chunk(sc):
            # v for s-chunk sc: psum [128 tokens, 256], evicted into va[sc].
            ps = ps_mm.tile([128, 512], F32, tag="mm", name="psv")
            for c in range(NDI):
                nc.tensor.matmul(
                    ps[:, 0:256],
                    lhsT=hst[:, c, sc * KC:(sc + 1) * KC],
                    rhs=wvt[:, c, :],
                    start=(c == 0),
                    stop=(with_bias is False and c == NDI - 1),
                )
            if with_bias:
                nc.tensor.matmul(
                    ps[:, 0:256],
                    lhsT=ones[0:1, 0:128],
                    rhs=bvt[0:1, :],
                    start=False,
                    stop=True,
                )
            nc.vector.tensor_copy(va[sc][:], ps[:, 0:256])

        def emit_score_exp(h, j, c):
            # transposed scores sT[k, q] for k-chunk c, both softmax halves,
            # then exp on ACT -> bf16 et tile [128, 2*QB].
            sp = ps_sc.tile([128, 2 * QB], F32, tag="sp", name="sp")
            for half in range(2):
                nc.tensor.matmul(
                    sp[:, half * QB:(half + 1) * QB],
                    lhsT=kT[h][half * 64:(half + 1) * 64, c * KC:(c + 1) * KC],
                    rhs=qT[h][half * 64:(half + 1) * 64, j * QB:(j + 1) * QB],
                    start=True,
                    stop=True,
                )
            et = epool.tile([128, 2 * QB], BF16, tag="et", name="et")
            nc.scalar.activation(
                et[:],
                sp[:],
                mybir.ActivationFunctionType.Exp,
                bias=maskt[:, c:c + 1],
                scale=float(HD) ** -0.5,
            )
            return et

        def emit_pv_chunk(h, c, et, pvt, smt):
            # PV + softmax-sum accumulation for one k-chunk: 8 64-row matmuls
            # into the packed pv bank, 8 1-row ones-matmuls into the sums
            # bank.  Only the first matmul of c==0 uses start=True per bank
            # (2KB zero-region covers the rest of the round).
            first, last = c == 0, c == NKC - 1
            for qc in range(NQC):
                for half in range(2):
                    sl = et[:, half * QB + qc * 128:half * QB + (qc + 1) * 128]
                    r = 2 * qc + half
                    nc.tensor.matmul(
                        pvt[:, r * HD:(r + 1) * HD],
                        lhsT=sl,
                        rhs=va[c][:, h * HD:(h + 1) * HD],
                        start=(first and r == 0),
                        stop=last,
                        skip_group_check=True,
                    )
                    nc.tensor.matmul(
                        smt[:, r:r + 1],
                        lhsT=sl,
                        rhs=ones[:, 0:1],
                        start=(first and r == 0),
                        stop=last,
                        skip_group_check=True,
                    )

        def emit_combine(h, j, pvt, smt):
            # stg[q, d] = pv1/sum1 - lam * pv2/sum2 per q-subchunk, then PE
            # transpose (plain matmul vs identity) into [d, q] stacked by
            # head parity, one DVE eviction per (h, j) into stgT.
            rt = gpool.tile([128, 8], F32, tag="rt", name="rt")
            nc.vector.reciprocal(out=rt[:], in_=smt[:, 0:8])
            trp = ps_mm.tile([128, 512], F32, tag="mm", name="trp")
            hh = (h % 2) * 64
            for qc in range(NQC):
                t2 = gpool.tile([128, HD], F32, tag="t2", name="t2")
                nc.vector.tensor_scalar(
                    out=t2[:],
                    in0=pvt[:, (2 * qc + 1) * HD:(2 * qc + 2) * HD],
                    scalar1=rt[:, 2 * qc + 1:2 * qc + 2],
                    scalar2=float(lam),
                    op0=mult,
                    op1=mult,
                )
                stg = gpool.tile([128, HD], BF16, tag="stg", name="stg")
                nc.vector.scalar_tensor_tensor(
                    out=stg[:],
                    in0=pvt[:, 2 * qc * HD:(2 * qc + 1) * HD],
                    scalar=rt[:, 2 * qc:2 * qc + 1],
                    in1=t2[:],
                    op0=mult,
                    op1=subtract,
                )
                nc.tensor.matmul(
                    trp[hh:hh + 64, qc * 128:(qc + 1) * 128],
                    lhsT=stg[:],
                    rhs=ident[:],
                    start=(qc == 0),
                    stop=True,
                    skip_group_check=True,
                )
            nc.vector.tensor_copy(
                stgT[h // 2][hh:hh + 64, j * QB:(j + 1) * QB],
                trp[hh:hh + 64, :],
            )

        def emit_outproj_chunk(j, d):
            # partial out-projection: outT[dout chunk, q block], K=128 per
            # stacked head-pair; fp16 partials DMA'd out on the SP queue.
            ps = ps_mm.tile([128, 512], F32, tag="mm", name="pso")
            for p in range(HPC // 2):
                nc.tensor.matmul(
                    ps[:],
                    lhsT=wot[p][:, d * 128:(d + 1) * 128],
                    rhs=stgT[p][:, j * QB:(j + 1) * QB],
                    start=(p == 0),
                    stop=(p == HPC // 2 - 1),
                )
            ot = opool.tile([128, 512], F16, tag="ot", name="ot")
            nc.vector.tensor_copy(ot[:], ps[:])
            nc.sync.dma_start(
                out=out_d[d * 128:(d + 1) * 128, j * QB:(j + 1) * QB],
                in_=ot[:],
            )

        # ---- emission schedule ----
        # Primary stream: score chunks (they feed ACT, the bottleneck).
        # Fillers drain between chunks: v-projection first, then deferred PV
        # rounds (per-chunk granularity, rounds kept contiguous so the packed
        # pv bank only ever holds one accumulation round), next head's
        # projections (usually JIT-emitted and no-op'd here), out-proj.
        for _rep in range(repeat):
            fillers = deque()
            state = {"vp": 0, "pv": 0}
            for sc in range(NKC):
                fillers.append(("vp", lambda sc=sc: emit_vproj_chunk(sc)))

            def drain(n):
                for _ in range(n):
                    if not fillers:
                        return
                    kind, thunk = fillers.popleft()
                    thunk()
                    if kind in state:
                        state[kind] += 1 if kind == "vp" else -1

            backlog = []  # (h, j, [et tiles]) with PV not yet emitted

            def release_block(bh, bj, bets):
                holder = {}

                def pvchunk(c):
                    if not holder:
                        holder["pv"] = ps_pv.tile(
                            [128, 8 * HD], F32, tag="pv", name="pvt"
                        )
                        holder["sm"] = ps_sm.tile([128, 16], F32, tag="sm", name="smt")
                    emit_pv_chunk(bh, c, bets[c], holder["pv"], holder["sm"])

                for c in range(NKC):
                    fillers.append(("pv", lambda c=c: pvchunk(c)))
                fillers.append(
                    ("pv", lambda: emit_combine(bh, bj, holder["pv"], holder["sm"]))
                )
                state["pv"] += NKC + 1

            emit_qkproj_group("k", 0, 0)
            emit_qkproj_group("q", 0, 0)

            for h in range(HPC):
                for j in range(NJ):
                    emit_qkproj_group("q", h, j)
                    # the packed pv bank admits one round at a time: flush any
                    # deferred round before opening an inline one
                    while state["pv"]:
                        drain(1)
                    inline = state["vp"] == NKC and not backlog
                    if inline:
                        pvt = ps_pv.tile([128, 8 * HD], F32, tag="pv", name="pvt")
                        smt = ps_sm.tile([128, 16], F32, tag="sm", name="smt")
                    ets = []
                    for c in range(NKC):
                        emit_qkproj_group("k", h, c // NQC)
                        et = emit_score_exp(h, j, c)
                        ets.append(et)
                        if inline:
                            emit_pv_chunk(h, c, et, pvt, smt)
                        drain(1)
                    if inline:
                        emit_combine(h, j, pvt, smt)
                    else:
                        backlog.append((h, j, ets))
                        if state["vp"] == NKC:
                            for blk in backlog:
                                release_block(*blk)
                            backlog = []
                    # queue next head's projections (JIT-covered fallbacks)
                    if j == 0 and h + 1 < HPC:
                        for g in range(NJ):
                            fillers.append(
                                ("kp", lambda a=h + 1, b=g:
                                 emit_qkproj_group("k", a, b))
                            )
                        fillers.append(
                            ("qp", lambda a=h + 1: emit_qkproj_group("q", a, 0))
                        )
                    # out-projection for block j once the last head finished it
                    if h == HPC - 1:
                        for d in range(NDI):
                            fillers.append(
                                ("op", lambda a=j, b=d: emit_outproj_chunk(a, b))
                            )
            drain(len(fillers))

    nc.compile()
    return nc


def _prep_inputs(hidden_states, attention_mask, Wq, bq, Wk, bk, Wv, bv, Wo):
    """Build the 8 per-core input maps (host-side shard + transpose + cast)."""
    in_maps = []
    hsT = [np.ascontiguousarray(hidden_states[b].T).astype(npbf16) for b in range(B)]
    maskc = [
        np.ascontiguousarray(
            ((1.0 - attention_mask[b]) * -10000.0).astype(np.float32).reshape(NKC, KC).T
        )
        for b in range(B)
    ]
    ident = np.eye(128, dtype=npbf16)
    for core in range(NCORES):
        b = core // (NCORES // B)
        hb = (core % (NCORES // B)) * HPC
        heads = range(hb, hb + HPC)
        qk_idx = np.concatenate(
            [np.r_[h * HD:(h + 1) * HD, D + h * HD:D + (h + 1) * HD] for h in heads]
        )
        v_idx = np.r_[hb * HD:(hb + HPC) * HD]
        in_maps.append(
            {
                "hst": hsT[b],
                "wq": np.ascontiguousarray(Wq[:, qk_idx]).astype(npbf16),
                "wk": np.ascontiguousarray(Wk[:, qk_idx]).astype(npbf16),
                "wv": np.ascontiguousarray(Wv[:, v_idx]).astype(npbf16),
                "wo": np.ascontiguousarray(Wo[v_idx, :]).astype(npbf16),
                "bq": bq[qk_idx].reshape(1, -1).astype(npbf16),
                "bk": bk[qk_idx].reshape(1, -1).astype(npbf16),
                "bv": bv[v_idx].reshape(1, -1).astype(npbf16),
                "maskc": maskc[b],
                "ident": ident,
            }
        )
    return in_maps


def kernel(
    hidden_states,
    attention_mask,
    Wq,
    bq,
    Wk,
    bk,
    Wv,
    bv,
    Wo,
    bo,
    lq1,
    lk1,
    lq2,
    lk2,
):
    global LAST_RESULTS
    args = [hidden_states, attention_mask, Wq, bq, Wk, bk, Wv, bv, Wo, bo]
    hidden_states, attention_mask, Wq, bq, Wk, bk, Wv, bv, Wo, bo = (
        np.asarray(a, dtype=np.float32) for a in args
    )
    lq1, lk1, lq2, lk2 = (np.asarray(a, dtype=np.float64) for a in (lq1, lk1, lq2, lk2))
    lam = float(np.exp(lq1 @ lk1) - np.exp(lq2 @ lk2) + 0.8)

    with_bias = not (
        np.all(bq == 0) and np.all(bk == 0) and np.all(bv == 0)
    )
    key = (round(lam, 9), with_bias)
    if key not in _BUILD_CACHE:
        _BUILD_CACHE.clear()
        _BUILD_CACHE[key] = _build(lam, with_bias)
    nc = _BUILD_CACHE[key]

    in_maps = _prep_inputs(hidden_states, attention_mask, Wq, bq, Wk, bk, Wv, bv, Wo)
    res = run_bass_kernel_spmd(nc, in_maps, core_ids=list(range(NCORES)), trace=TRACE)
    LAST_RESULTS = res

    out = np.empty((B, S, D), dtype=np.float32)
    gpb = NCORES // B
    for b in range(B):
        acc = res.results[b * gpb]["outT"].astype(np.float32)
        for g in range(1, gpb):
            acc = acc + res.results[b * gpb + g]["outT"]
        out[b] = acc.T + bo[None, :]
    return out


# revision 23
# speedup vs baseline: 1.2891x; 1.2891x over previous
"""Differential attention kernel for Trainium2, 8-core SPMD.

Problem: B=2, S=2048, D=1024, 16 heads x 64 head-dim differential attention
(two softmaxes combined with a scalar lambda), with input/output projections.

Sharding: data-parallel over batch (2 groups of 4 cores) x tensor-parallel
over heads (4 heads per core). Each core computes q/k/v projections for its
4 heads, both attention softmaxes, and a partial output projection
(its heads' rows of Wo). Host sums the 4 partial outputs per batch, adds bo.

Design (driven by the TimelineSim cost model, where a matmul costs
out-free-size x pe_cycle and an ACT op costs free-size x act_cycle):
  - Projections produce transposed q/k ([dh 128 = q1|q2 stacked, S]) and
    v ([s, 4*64]) with K=128/M=128 matmuls (row-optimal).
  - Scores are computed transposed, sT[k, q], per 128-token k-chunk:
    two K=64 matmuls (halves on partition ranges 0:64 / 64:128).
  - exp() runs on ACT straight out of PSUM ([128, 1024] per chunk,
    double-buffered PSUM), mask folded into the per-partition bias and the
    1/sqrt(hd) scale into the activation scale.  ACT is the kernel's
    bottleneck engine (~266 us busy), so nothing else runs on ACT.
  - PV is orientation-flipped vs the baseline: out[q, d] with lhsT = et
    chunk [k 128, q 128] and rhs = v [k 128, d 64] -> 64-row matmuls at
    full K=128/M=128 (half the PE rows of the [d, q] orientation).  All
    8 accumulators (4 q-subchunks x 2 softmax halves) pack into ONE psum
    bank; softmax sums accumulate via 1-row ones-matmuls into a second
    bank.  PSUM 2KB zero-region semantics: only the first matmul touching
    a bank per round uses start=True, the other regions' first writes
    clear their pending-zero bytes (fresh write), later writes accumulate.
  - Normalization needs no partition broadcast in this orientation:
    DVE reciprocal of the sums column + per-partition tensor_scalar ops
    combine the halves with lambda folded in.
  - stg [q, d] is PE-transposed (plain bf16 matmul against an identity)
    into [d, q] with head pairs stacked on partitions, so the output
    projection contracts K=128 (half the PE rows of per-head K=64).
All matmuls run in bf16 with fp32 PSUM accumulation; output partials ship
as fp16 and are reduced across cores in fp32 on the host.

Engine budget per core (cost model): ACT 266us (256 exp ops), PE ~255us
(~608k matmul rows), DVE ~90us.  The emission order keeps ACT dense:
score chunks are the primary stream; projection groups are emitted
just-in-time ahead of their consumers; PV work for early (h, j) blocks is
deferred (et tiles are held in SBUF) until the v-projection has drained,
then drains as filler between score chunks.
"""

import sys

sys.path.insert(0, "/opt/trn_rl_repo")

from collections import deque
from contextlib import ExitStack

import ml_dtypes
import numpy as np

import concourse.bacc as bacc
import concourse.tile as tile
from concourse import mybir
from concourse.bass_utils import run_bass_kernel_spmd

B, S, D = 2, 2048, 1024
NH, HD = 16, 64
NCORES = 8
HPC = 4              # heads per core
QB = 512             # q block (free dim of score matmuls)
NJ = S // QB         # 4
KC = 128             # k chunk (partition dim of transposed scores)
NKC = S // KC        # 16
NDI = D // 128       # 8 contraction chunks for projections
NQC = QB // 128      # 4 q-subchunks per block (PV output partition tiles)

BF16 = mybir.dt.bfloat16
F32 = mybir.dt.float32
F16 = mybir.dt.float16
npbf16 = ml_dtypes.bfloat16

_BUILD_CACHE = {}
TRACE = False
LAST_RESULTS = None


def _build(lam: float, with_bias: bool = True, repeat: int = 1):
    nc = bacc.Bacc(None, target_bir_lowering=False)
    mult = mybir.AluOpType.mult
    subtract = mybir.AluOpType.subtract

    hst_d = nc.dram_tensor("hst", [D, S], BF16, kind="ExternalInput")
    wq_d = nc.dram_tensor("wq", [D, 2 * HPC * HD], BF16, kind="ExternalInput")
    wk_d = nc.dram_tensor("wk", [D, 2 * HPC * HD], BF16, kind="ExternalInput")
    wv_d = nc.dram_tensor("wv", [D, HPC * HD], BF16, kind="ExternalInput")
    wo_d = nc.dram_tensor("wo", [HPC * HD, D], BF16, kind="ExternalInput")
    bq_d = nc.dram_tensor("bq", [1, 2 * HPC * HD], BF16, kind="ExternalInput")
    bk_d = nc.dram_tensor("bk", [1, 2 * HPC * HD], BF16, kind="ExternalInput")
    bv_d = nc.dram_tensor("bv", [1, HPC * HD], BF16, kind="ExternalInput")
    mask_d = nc.dram_tensor("maskc", [KC, NKC], F32, kind="ExternalInput")
    id_d = nc.dram_tensor("ident", [128, 128], BF16, kind="ExternalInput")
    out_d = [
        nc.dram_tensor(f"outT{p}", [D, S], F16, kind="ExternalOutput")
        for p in range(HPC // 2)
    ]

    with tile.TileContext(nc) as tc, ExitStack() as ctx:
        const = ctx.enter_context(tc.tile_pool(name="const", bufs=1))
        wpool = ctx.enter_context(tc.tile_pool(name="wpool", bufs=1))
        hpool = ctx.enter_context(tc.tile_pool(name="hpool", bufs=1))
        qkpool = ctx.enter_context(tc.tile_pool(name="qkpool", bufs=1))
        vpool = ctx.enter_context(tc.tile_pool(name="vpool", bufs=1))
        epool = ctx.enter_context(tc.tile_pool(name="epool", bufs=38))
        spool = ctx.enter_context(tc.tile_pool(name="spool", bufs=1))
        gpool = ctx.enter_context(tc.tile_pool(name="gpool", bufs=8))
        opool = ctx.enter_context(tc.tile_pool(name="opool", bufs=6))
        ps_mm = ctx.enter_context(tc.tile_pool(name="ps_mm", bufs=2, space="PSUM"))
        ps_sc = ctx.enter_context(tc.tile_pool(name="ps_sc", bufs=2, space="PSUM"))
        ps_pv = ctx.enter_context(tc.tile_pool(name="ps_pv", bufs=1, space="PSUM"))
        ps_sm = ctx.enter_context(tc.tile_pool(name="ps_sm", bufs=1, space="PSUM"))

        # ---- input DMAs.  The cost model serializes all DMA transfers on one
        # device (~324 GB/s), so order them by first use: head0's wk/wq
        # slices + hs j0 (unblocks the first score chunks ~6.5us in), then
        # mask, the remaining hs j-blocks (kproj JIT at chunks 4/8/12), wv,
        # the other heads' wk/wq, identity, wo.  One queue (Pool SWDGE) keeps
        # the global order deterministic and off the ACT/DVE engines. ----
        wkt = wpool.tile([128, NDI, 512], BF16, tag="wk", name="wkt")
        wqt = wpool.tile([128, NDI, 512], BF16, tag="wq", name="wqt")
        wvt = wpool.tile([128, NDI, 256], BF16, tag="wv", name="wvt")
        hst = hpool.tile([128, NDI, S], BF16, tag="hs", name="hst")
        wk_r = wk_d[:].rearrange("(c p) m -> p c m", p=128)
        wq_r = wq_d[:].rearrange("(c p) m -> p c m", p=128)
        wv_r = wv_d[:].rearrange("(c p) m -> p c m", p=128)
        hs_r = hst_d[:].rearrange("(c p) s -> p c s", p=128)
        maskt = const.tile([KC, NKC], F32, tag="mask")
        ident = const.tile([128, 128], BF16, tag="ident")
        ones = const.tile([128, QB], BF16, tag="ones")
        nc.gpsimd.memset(ones[:], 1.0)
        wot = [wpool.tile([128, D], BF16, tag=f"wo{p}", name=f"wo{p}")
               for p in range(HPC // 2)]

        nc.gpsimd.dma_start(out=wkt[:, :, 0:128], in_=wk_r[:, :, 0:128])
        nc.gpsimd.dma_start(out=hst[:, 0:4, 0:QB], in_=hs_r[:, 0:4, 0:QB])
        nc.gpsimd.dma_start(out=hst[:, 4:8, 0:QB], in_=hs_r[:, 4:8, 0:QB])
        nc.gpsimd.dma_start(out=wqt[:, :, 0:128], in_=wq_r[:, :, 0:128])
        nc.gpsimd.dma_start(out=maskt[:], in_=mask_d[:])
        for j in range(1, NJ):
            nc.gpsimd.dma_start(
                out=hst[:, :, j * QB:(j + 1) * QB], in_=hs_r[:, :, j * QB:(j + 1) * QB]
            )
        nc.gpsimd.dma_start(out=wvt[:], in_=wv_r[:])
        nc.gpsimd.dma_start(out=wkt[:, :, 128:512], in_=wk_r[:, :, 128:512])
        nc.gpsimd.dma_start(out=wqt[:, :, 128:512], in_=wq_r[:, :, 128:512])
        nc.gpsimd.dma_start(out=ident[:], in_=id_d[:])
        for p in range(HPC // 2):
            nc.gpsimd.dma_start(out=wot[p][:], in_=wo_d[p * 128:(p + 1) * 128, :])
        if with_bias:
            bqt = const.tile([1, 2 * HPC * HD], BF16, tag="bq")
            nc.gpsimd.dma_start(out=bqt[:], in_=bq_d[:])
            bkt = const.tile([1, 2 * HPC * HD], BF16, tag="bk")
            nc.gpsimd.dma_start(out=bkt[:], in_=bk_d[:])
            bvt = const.tile([1, HPC * HD], BF16, tag="bv")
            nc.gpsimd.dma_start(out=bvt[:], in_=bv_d[:])

        # ---- persistent per-head tiles ----
        qT = [qkpool.tile([128, S], BF16, tag=f"qT{h}", name=f"qT{h}")
              for h in range(HPC)]
        kT = [qkpool.tile([128, S], BF16, tag=f"kT{h}", name=f"kT{h}")
              for h in range(HPC)]
        va = [vpool.tile([128, HPC * HD], BF16, tag=f"va{c}", name=f"va{c}")
              for c in range(NKC)]
        stgT = [spool.tile([128, S], BF16, tag=f"sT{p}", name=f"sT{p}")
                for p in range(HPC // 2)]

        kdone = [[False] * NJ for _ in range(HPC)]
        qdone = [[False] * NJ for _ in range(HPC)]

        def emit_qkproj_group(which, h, g):
            # one j-group of the q or k projection for head h: psum [128, 512]
            # (partitions = q1|q2 of the head), evicted bf16 into qT/kT.
            wt, bt, dst, done = (
                (wqt, bqt if with_bias else None, qT[h], qdone)
                if which == "q"
                else (wkt, bkt if with_bias else None, kT[h], kdone)
            )
            if done[h][g]:
                return
            done[h][g] = True
            # medium-high priority: the eviction feeds upcoming score chunks,
            # so it must beat combine/out-proj work on PE and DVE
            with tc.high_priority(offset=1500):
                ps = ps_mm.tile([128, 512], F32, tag="mm", name="psqk")
                for c in range(NDI):
                    nc.tensor.matmul(
                        ps[:],
                        lhsT=wt[:, c, h * 128:(h + 1) * 128],
                        rhs=hst[:, c, g * QB:(g + 1) * QB],
                        start=(c == 0),
                        stop=(with_bias is False and c == NDI - 1),
                    )
                if with_bias:
                    nc.tensor.matmul(
                        ps[:],
                        lhsT=bt[0:1, h * 128:(h + 1) * 128],
                        rhs=ones[0:1, :],
                        start=False,
                        stop=True,
                    )
                nc.vector.tensor_copy(dst[:, g * QB:(g + 1) * QB], ps[:])

        def emit_vproj_chunk(sc):
            # v for s-chunk sc: psum [128 tokens, 256], evicted into va[sc].
            ps = ps_mm.tile([128, 512], F32, tag="mm", name="psv")
            for c in range(NDI):
                nc.tensor.matmul(
                    ps[:, 0:256],
                    lhsT=hst[:, c, sc * KC:(sc + 1) * KC],
                    rhs=wvt[:, c, :],
                    start=(c == 0),
                    stop=(with_bias is False and c == NDI - 1),
                )
            if with_bias:
                nc.tensor.matmul(
                    ps[:, 0:256],
                    lhsT=ones[0:1, 0:128],
                    rhs=bvt[0:1, :],
                    start=False,
                    stop=True,
                )
            nc.vector.tensor_copy(va[sc][:], ps[:, 0:256])

        def emit_score_exp(h, j, c):
            # transposed scores sT[k, q] for k-chunk c, both softmax halves,
            # then exp on ACT -> bf16 et tile [128, 2*QB].
            # top priority: every score chunk gates an ACT exp, and ACT is the
            # bottleneck engine — scores must never lose PE to filler work
            with tc.high_priority(offset=4000):
                sp = ps_sc.tile([128, 2 * QB], F32, tag="sp", name="sp")
                for half in range(2):
                    nc.tensor.matmul(
                        sp[:, half * QB:(half + 1) * QB],
                        lhsT=kT[h][half * 64:(half + 1) * 64, c * KC:(c + 1) * KC],
                        rhs=qT[h][half * 64:(half + 1) * 64, j * QB:(j + 1) * QB],
                        start=True,
                        stop=True,
                    )
                et = epool.tile([128, 2 * QB], BF16, tag="et", name="et")
                nc.scalar.activation(
                    et[:],
                    sp[:],
                    mybir.ActivationFunctionType.Exp,
                    bias=maskt[:, c:c + 1],
                    scale=float(HD) ** -0.5,
                )
            return et

        def emit_pv_chunk(h, c, et, pvt, smt):
            # PV + softmax-sum accumulation for one k-chunk: 8 64-row matmuls
            # into the packed pv bank, 8 1-row ones-matmuls into the sums
            # bank.  Only the first matmul of c==0 uses start=True per bank
            # (2KB zero-region covers the rest of the round).
            first, last = c == 0, c == NKC - 1
            for qc in range(NQC):
                for half in range(2):
                    sl = et[:, half * QB + qc * 128:half * QB + (qc + 1) * 128]
                    r = 2 * qc + half
                    nc.tensor.matmul(
                        pvt[:, r * HD:(r + 1) * HD],
                        lhsT=sl,
                        rhs=va[c][:, h * HD:(h + 1) * HD],
                        start=(first and r == 0),
                        stop=last,
                        skip_group_check=True,
                    )
                    nc.tensor.matmul(
                        smt[:, r:r + 1],
                        lhsT=sl,
                        rhs=ones[:, 0:1],
                        start=(first and r == 0),
                        stop=last,
                        skip_group_check=True,
                    )

        def emit_combine(h, j, pvt, smt):
            # stg[q, d] = pv1/sum1 - lam * pv2/sum2 per q-subchunk, then PE
            # transpose (plain matmul vs identity) into [d, q] stacked by
            # head parity, one DVE eviction per (h, j) into stgT.
            rt = gpool.tile([128, 8], F32, tag="rt", name="rt")
            nc.vector.reciprocal(out=rt[:], in_=smt[:, 0:8])
            trp = ps_mm.tile([128, 512], F32, tag="mm", name="trp")
            hh = (h % 2) * 64
            # all t2 ops first, then the stg ops: halves the serial DVE chain
            t2s = []
            for qc in range(NQC):
                t2 = gpool.tile([128, HD], F32, tag="t2", name="t2")
                nc.vector.tensor_scalar(
                    out=t2[:],
                    in0=pvt[:, (2 * qc + 1) * HD:(2 * qc + 2) * HD],
                    scalar1=rt[:, 2 * qc + 1:2 * qc + 2],
                    scalar2=float(lam),
                    op0=mult,
                    op1=mult,
                )
                t2s.append(t2)
            for qc in range(NQC):
                stg = gpool.tile([128, HD], BF16, tag="stg", name="stg")
                nc.vector.scalar_tensor_tensor(
                    out=stg[:],
                    in0=pvt[:, 2 * qc * HD:(2 * qc + 1) * HD],
                    scalar=rt[:, 2 * qc:2 * qc + 1],
                    in1=t2s[qc][:],
                    op0=mult,
                    op1=subtract,
                )
                nc.tensor.matmul(
                    trp[hh:hh + 64, qc * 128:(qc + 1) * 128],
                    lhsT=stg[:],
                    rhs=ident[:],
                    start=(qc == 0),
                    stop=True,
                    skip_group_check=True,
                )
            nc.vector.tensor_copy(
                stgT[h // 2][hh:hh + 64, j * QB:(j + 1) * QB],
                trp[hh:hh + 64, :],
            )

        def emit_outproj_chunk(j, dd, p, use_act=False):
            # per-head-pair partial out-projection for a PAIR of dout chunks
            # (dd = 0..3 -> douts 2dd, 2dd+1): two K=128 matmuls, two
            # evictions into one [128, 1024] tile, ONE output DMA (SP DMA
            # triggers cost 565ns of sequencer each — batching halves them).
            # The host sums the two pair partials per core.  The very last
            # block's evictions alternate onto ACT (idle once exps are done).
            ot = opool.tile([128, 1024], F16, tag="ot", name="ot")
            for i in range(2):
                d = 2 * dd + i
                ps = ps_mm.tile([128, 512], F32, tag="mm", name="pso")
                nc.tensor.matmul(
                    ps[:],
                    lhsT=wot[p][:, d * 128:(d + 1) * 128],
                    rhs=stgT[p][:, j * QB:(j + 1) * QB],
                    start=True,
                    stop=True,
                )
                if use_act and i % 2 == 0:
                    nc.scalar.copy(ot[:, i * 512:(i + 1) * 512], ps[:])
                else:
                    nc.vector.tensor_copy(ot[:, i * 512:(i + 1) * 512], ps[:])
            dst = out_d[p][2 * dd * 128:(2 * dd + 2) * 128,
                           j * QB:(j + 1) * QB].rearrange("(d p) s -> p d s", p=128)
            nc.sync.dma_start(
                out=dst, in_=ot[:].rearrange("p (d s) -> p d s", s=QB)
            )

        # ---- emission schedule ----
        # Primary stream: score chunks (they feed ACT, the bottleneck).
        # Fillers drain between chunks: v-projection first, then deferred PV
        # rounds (per-chunk granularity, rounds kept contiguous so the packed
        # pv bank only ever holds one accumulation round), next head's
        # projections (usually JIT-emitted and no-op'd here), out-proj.
        for _rep in range(repeat):
            fillers = deque()
            state = {"vp": 0, "pv": 0}
            for sc in range(NKC):
                fillers.append(("vp", lambda sc=sc: emit_vproj_chunk(sc)))

            def drain(n):
                for _ in range(n):
                    if not fillers:
                        return
                    kind, thunk = fillers.popleft()
                    thunk()
                    if kind in state:
                        state[kind] += 1 if kind == "vp" else -1

            backlog = []  # (h, j, [et tiles]) with PV not yet emitted

            def release_block(bh, bj, bets):
                holder = {}

                def pvchunk(c):
                    if not holder:
                        holder["pv"] = ps_pv.tile(
                            [128, 8 * HD], F32, tag="pv", name="pvt"
                        )
                        holder["sm"] = ps_sm.tile([128, 16], F32, tag="sm", name="smt")
                    emit_pv_chunk(bh, c, bets[c], holder["pv"], holder["sm"])

                for c in range(NKC):
                    fillers.append(("pv", lambda c=c: pvchunk(c)))
                fillers.append(
                    ("pv", lambda: emit_combine(bh, bj, holder["pv"], holder["sm"]))
                )
                state["pv"] += NKC + 1

            emit_qkproj_group("k", 0, 0)
            emit_qkproj_group("q", 0, 0)
            chunk_no = 0

            for h in range(HPC):
                for j in range(NJ):
                    emit_qkproj_group("q", h, j)
                    # prefetch upcoming q-projections near the front of the
                    # filler queue so their evictions land before those
                    # blocks' first scores (kills the block-boundary ACT gap)
                    bi = h * NJ + j
                    for nb in (bi + 2, bi + 1):
                        nh, njx = divmod(nb, NJ)
                        if nh < HPC:
                            fillers.appendleft(
                                ("qp", lambda a=nh, b=njx:
                                 emit_qkproj_group("q", a, b))
                            )
                    # next head's k-projection groups ahead of its first block
                    if j == 1 and h + 1 < HPC:
                        for g in reversed(range(NJ)):
                            fillers.appendleft(
                                ("kp", lambda a=h + 1, b=g:
                                 emit_qkproj_group("k", a, b))
                            )
                    # inline PV only once v-proj is done and no deferred round
                    # is still queued (the packed pv bank admits one round at
                    # a time; deferred rounds drain with priority below)
                    inline = state["vp"] == NKC and not backlog
                    holder = {}

                    def own_pv(c, h=h):
                        if not holder:
                            holder["pv"] = ps_pv.tile(
                                [128, 8 * HD], F32, tag="pv", name="pvt"
                            )
                            holder["sm"] = ps_sm.tile(
                                [128, 16], F32, tag="sm", name="smt"
                            )
                        emit_pv_chunk(h, c, ets[c], holder["pv"], holder["sm"])

                    ets = []
                    pend = deque()
                    for c in range(NKC):
                        emit_qkproj_group("k", h, c // NQC)
                        ets.append(emit_score_exp(h, j, c))
                        chunk_no += 1
                        if inline:
                            pend.append(c)
                        if state["pv"]:
                            # finish the deferred round first (bank exclusive)
                            drain(2)
                        elif inline and len(pend) > 2:
                            # lag-2 pipeline: PV trails the exp by two chunks
                            while len(pend) > 2:
                                own_pv(pend.popleft())
                            drain(1)
                        elif fillers and fillers[0][0] == "vp":
                            # spread the v-projection over two blocks
                            if chunk_no % 2:
                                drain(1)
                        else:
                            drain(1)
                    if inline:
                        # the packed pv bank admits one round at a time: any
                        # deferred round must fully emit (incl. its combine)
                        # before this block's round opens
                        while state["pv"]:
                            drain(1)
                        while pend:
                            own_pv(pend.popleft())
                        emit_combine(h, j, holder["pv"], holder["sm"])
                    else:
                        backlog.append((h, j, ets))
                        if state["vp"] == NKC:
                            for blk in backlog:
                                release_block(*blk)
                            backlog = []
                    # pair-p out-projection for block j once its second head
                    # finished the block (h==1 -> pair 0, h==3 -> pair 1)
                    if h % 2 == 1:
                        last = h == HPC - 1 and j == NJ - 1
                        for dd in range(NDI // 2):
                            fillers.append(
                                ("op", lambda a=j, b=dd, c2=h // 2, ua=last:
                                 emit_outproj_chunk(a, b, c2, use_act=ua))
                            )
            drain(len(fillers))

    nc.compile()
    return nc


def _prep_inputs(hidden_states, attention_mask, Wq, bq, Wk, bk, Wv, bv, Wo):
    """Build the 8 per-core input maps (host-side shard + transpose + cast)."""
    in_maps = []
    hsT = [np.ascontiguousarray(hidden_states[b].T).astype(npbf16) for b in range(B)]
    maskc = [
        np.ascontiguousarray(
            ((1.0 - attention_mask[b]) * -10000.0).astype(np.float32).reshape(NKC, KC).T
        )
        for b in range(B)
    ]
    ident = np.eye(128, dtype=npbf16)
    for core in range(NCORES):
        b = core // (NCORES // B)
        hb = (core % (NCORES // B)) * HPC
        heads = range(hb, hb + HPC)
        qk_idx = np.concatenate(
            [np.r_[h * HD:(h + 1) * HD, D + h * HD:D + (h + 1) * HD] for h in heads]
        )
        v_idx = np.r_[hb * HD:(hb + HPC) * HD]
        in_maps.append(
            {
                "hst": hsT[b],
                "wq": np.ascontiguousarray(Wq[:, qk_idx]).astype(npbf16),
                "wk": np.ascontiguousarray(Wk[:, qk_idx]).astype(npbf16),
                "wv": np.ascontiguousarray(Wv[:, v_idx]).astype(npbf16),
                "wo": np.ascontiguousarray(Wo[v_idx, :]).astype(npbf16),
                "bq": bq[qk_idx].reshape(1, -1).astype(npbf16),
                "bk": bk[qk_idx].reshape(1, -1).astype(npbf16),
                "bv": bv[v_idx].reshape(1, -1).astype(npbf16),
                "maskc": maskc[b],
                "ident": ident,
            }
        )
    return in_maps


def kernel(
    hidden_states,
    attention_mask,
    Wq,
    bq,
    Wk,
    bk,
    Wv,
    bv,
    Wo,
    bo,
    lq1,
    lk1,
    lq2,
    lk2,
):
    global LAST_RESULTS
    args = [hidden_states, attention_mask, Wq, bq, Wk, bk, Wv, bv, Wo, bo]
    hidden_states, attention_mask, Wq, bq, Wk, bk, Wv, bv, Wo, bo = (
        np.asarray(a, dtype=np.float32) for a in args
    )
    lq1, lk1, lq2, lk2 = (np.asarray(a, dtype=np.float64) for a in (lq1, lk1, lq2, lk2))
    lam = float(np.exp(lq1 @ lk1) - np.exp(lq2 @ lk2) + 0.8)

    with_bias = not (
        np.all(bq == 0) and np.all(bk == 0) and np.all(bv == 0)
    )
    key = (round(lam, 9), with_bias)
    if key not in _BUILD_CACHE:
        _BUILD_CACHE.clear()
        _BUILD_CACHE[key] = _build(lam, with_bias)
    nc = _BUILD_CACHE[key]

    in_maps = _prep_inputs(hidden_states, attention_mask, Wq, bq, Wk, bk, Wv, bv, Wo)
    res = run_bass_kernel_spmd(nc, in_maps, core_ids=list(range(NCORES)), trace=TRACE)
    LAST_RESULTS = res

    out = np.empty((B, S, D), dtype=np.float32)
    gpb = NCORES // B
    for b in range(B):
        acc = res.results[b * gpb]["outT0"].astype(np.float32)
        acc += res.results[b * gpb]["outT1"]
        for g in range(1, gpb):
            acc += res.results[b * gpb + g]["outT0"]
            acc += res.results[b * gpb + g]["outT1"]
        out[b] = acc.T + bo[None, :]
    return out


# revision 57
# speedup vs baseline: 1.3258x; 1.0285x over previous
"""Differential attention kernel for Trainium2, 8-core SPMD.

Problem: B=2, S=2048, D=1024, 16 heads x 64 head-dim differential attention
(two softmaxes combined with a scalar lambda), with input/output projections.

Sharding: data-parallel over batch (2 groups of 4 cores) x tensor-parallel
over heads (4 heads per core). Each core computes q/k/v projections for its
4 heads, both attention softmaxes, and a partial output projection
(its heads' rows of Wo). Host sums the 4 partial outputs per batch, adds bo.

Design (driven by the TimelineSim cost model, where a matmul costs
out-free-size x pe_cycle and an ACT op costs free-size x act_cycle):
  - Projections produce transposed q/k ([dh 128 = q1|q2 stacked, S]) and
    v ([s, 4*64]) with K=128/M=128 matmuls (row-optimal).
  - Scores are computed transposed, sT[k, q], per 128-token k-chunk:
    two K=64 matmuls (halves on partition ranges 0:64 / 64:128).
  - exp() runs on ACT straight out of PSUM ([128, 1024] per chunk,
    double-buffered PSUM), mask folded into the per-partition bias and the
    1/sqrt(hd) scale into the activation scale.  ACT is the kernel's
    bottleneck engine (~266 us busy), so nothing else runs on ACT.
  - PV is orientation-flipped vs the baseline: out[q, d] with lhsT = et
    chunk [k 128, q 128] and rhs = v [k 128, d 64] -> 64-row matmuls at
    full K=128/M=128 (half the PE rows of the [d, q] orientation).  All
    8 accumulators (4 q-subchunks x 2 softmax halves) pack into ONE psum
    bank; softmax sums accumulate via 1-row ones-matmuls into a second
    bank.  PSUM 2KB zero-region semantics: only the first matmul touching
    a bank per round uses start=True, the other regions' first writes
    clear their pending-zero bytes (fresh write), later writes accumulate.
  - Normalization needs no partition broadcast in this orientation:
    DVE reciprocal of the sums column + per-partition tensor_scalar ops
    combine the halves with lambda folded in.
  - stg [q, d] is PE-transposed (plain bf16 matmul against an identity)
    into [d, q] with head pairs stacked on partitions, so the output
    projection contracts K=128 (half the PE rows of per-head K=64).
All matmuls run in bf16 with fp32 PSUM accumulation; output partials ship
as fp16 and are reduced across cores in fp32 on the host.

Engine budget per core (cost model): ACT 266us (256 exp ops), PE ~255us
(~608k matmul rows), DVE ~90us.  The emission order keeps ACT dense:
score chunks are the primary stream; projection groups are emitted
just-in-time ahead of their consumers; PV work for early (h, j) blocks is
deferred (et tiles are held in SBUF) until the v-projection has drained,
then drains as filler between score chunks.
"""

import sys

sys.path.insert(0, "/opt/trn_rl_repo")

from collections import deque
from contextlib import ExitStack

import ml_dtypes
import numpy as np

import concourse.bacc as bacc
import concourse.tile as tile
from concourse import mybir
from concourse.bass_utils import run_bass_kernel_spmd

B, S, D = 2, 2048, 1024
NH, HD = 16, 64
NCORES = 8
HPC = 4              # heads per core
QB = 512             # q block (free dim of score matmuls)
NJ = S // QB         # 4
KC = 128             # k chunk (partition dim of transposed scores)
NKC = S // KC        # 16
NDI = D // 128       # 8 contraction chunks for projections
NQC = QB // 128      # 4 q-subchunks per block (PV output partition tiles)

BF16 = mybir.dt.bfloat16
F32 = mybir.dt.float32
F16 = mybir.dt.float16
npbf16 = ml_dtypes.bfloat16

_BUILD_CACHE = {}
TRACE = False
LAST_RESULTS = None


def _build(lam: float, with_bias: bool = True, repeat: int = 1):
    nc = bacc.Bacc(None, target_bir_lowering=False)
    mult = mybir.AluOpType.mult
    subtract = mybir.AluOpType.subtract

    hst_d = nc.dram_tensor("hst", [D, S], BF16, kind="ExternalInput")
    wq_d = nc.dram_tensor("wq", [D, 2 * HPC * HD], BF16, kind="ExternalInput")
    wk_d = nc.dram_tensor("wk", [D, 2 * HPC * HD], BF16, kind="ExternalInput")
    wv_d = nc.dram_tensor("wv", [D, HPC * HD], BF16, kind="ExternalInput")
    wo_d = nc.dram_tensor("wo", [HPC * HD, D], BF16, kind="ExternalInput")
    bq_d = nc.dram_tensor("bq", [1, 2 * HPC * HD], BF16, kind="ExternalInput")
    bk_d = nc.dram_tensor("bk", [1, 2 * HPC * HD], BF16, kind="ExternalInput")
    bv_d = nc.dram_tensor("bv", [1, HPC * HD], BF16, kind="ExternalInput")
    mask_d = nc.dram_tensor("maskc", [KC, NKC], F32, kind="ExternalInput")
    id_d = nc.dram_tensor("ident", [128, 128], BF16, kind="ExternalInput")
    out_d = [
        nc.dram_tensor(f"outT{p}", [D, S], F16, kind="ExternalOutput")
        for p in range(HPC // 2)
    ]

    with tile.TileContext(nc) as tc, ExitStack() as ctx:
        const = ctx.enter_context(tc.tile_pool(name="const", bufs=1))
        wpool = ctx.enter_context(tc.tile_pool(name="wpool", bufs=1))
        hpool = ctx.enter_context(tc.tile_pool(name="hpool", bufs=1))
        qkpool = ctx.enter_context(tc.tile_pool(name="qkpool", bufs=1))
        vpool = ctx.enter_context(tc.tile_pool(name="vpool", bufs=1))
        epool = ctx.enter_context(tc.tile_pool(name="epool", bufs=42))
        spool = ctx.enter_context(tc.tile_pool(name="spool", bufs=1))
        gpool = ctx.enter_context(tc.tile_pool(name="gpool", bufs=8))
        opool = ctx.enter_context(tc.tile_pool(name="opool", bufs=6))
        ps_mm = ctx.enter_context(tc.tile_pool(name="ps_mm", bufs=2, space="PSUM"))
        ps_sc = ctx.enter_context(tc.tile_pool(name="ps_sc", bufs=2, space="PSUM"))
        ps_pv = ctx.enter_context(tc.tile_pool(name="ps_pv", bufs=1, space="PSUM"))
        ps_sm = ctx.enter_context(tc.tile_pool(name="ps_sm", bufs=1, space="PSUM"))

        # ---- input DMAs.  The cost model serializes all DMA transfers on one
        # device (~324 GB/s), so order them by first use: head0's wk/wq
        # slices + hs j0 (unblocks the first score chunks ~6.5us in), then
        # mask, the remaining hs j-blocks (kproj JIT at chunks 4/8/12), wv,
        # the other heads' wk/wq, identity, wo.  One queue (Pool SWDGE) keeps
        # the global order deterministic and off the ACT/DVE engines. ----
        wkt = wpool.tile([128, NDI, 512], BF16, tag="wk", name="wkt")
        wqt = wpool.tile([128, NDI, 512], BF16, tag="wq", name="wqt")
        wvt = wpool.tile([128, NDI, 256], BF16, tag="wv", name="wvt")
        hst = hpool.tile([128, NDI, S], BF16, tag="hs", name="hst")
        wk_r = wk_d[:].rearrange("(c p) m -> p c m", p=128)
        wq_r = wq_d[:].rearrange("(c p) m -> p c m", p=128)
        wv_r = wv_d[:].rearrange("(c p) m -> p c m", p=128)
        hs_r = hst_d[:].rearrange("(c p) s -> p c s", p=128)
        maskt = const.tile([KC, NKC], F32, tag="mask")
        ident = const.tile([128, 128], BF16, tag="ident")
        ones = const.tile([128, QB], BF16, tag="ones")
        nc.gpsimd.memset(ones[:], 1.0)
        wot = [wpool.tile([128, D], BF16, tag=f"wo{p}", name=f"wo{p}")
               for p in range(HPC // 2)]

        nc.sync.dma_start(out=wqt[:, :, 0:128], in_=wq_r[:, :, 0:128])
        nc.scalar.dma_start(out=hst[:, 0:4, 0:QB], in_=hs_r[:, 0:4, 0:QB])
        nc.sync.dma_start(out=hst[:, 4:8, 0:QB], in_=hs_r[:, 4:8, 0:QB])
        nc.sync.dma_start(out=wkt[:, :, 0:128], in_=wk_r[:, :, 0:128])
        nc.scalar.dma_start(out=maskt[:], in_=mask_d[:])
        for jh in range(2, 2 * NJ):  # j1..j3 in 256-column halves
            nc.sync.dma_start(
                out=hst[:, :, jh * 256:(jh + 1) * 256],
                in_=hs_r[:, :, jh * 256:(jh + 1) * 256],
            )
        nc.sync.dma_start(out=wvt[:], in_=wv_r[:])
        nc.sync.dma_start(out=ident[:], in_=id_d[:])
        nc.sync.dma_start(out=wkt[:, :, 128:512], in_=wk_r[:, :, 128:512])
        nc.sync.dma_start(out=wqt[:, :, 128:512], in_=wq_r[:, :, 128:512])
        for p in range(HPC // 2):
            nc.sync.dma_start(out=wot[p][:], in_=wo_d[p * 128:(p + 1) * 128, :])

        # PE p-state warm-up: ~3us of throwaway matmuls while the first DMAs
        # land, so the projection/score chain starts at full clock
        wps = ps_mm.tile([128, 512], F32, tag="mm", name="wps")
        for i in range(7):
            nc.tensor.matmul(
                wps[:], lhsT=ones[:, 0:128], rhs=ones[:],
                start=(i == 0), stop=(i == 6),
            )
        if with_bias:
            bqt = const.tile([1, 2 * HPC * HD], BF16, tag="bq")
            nc.gpsimd.dma_start(out=bqt[:], in_=bq_d[:])
            bkt = const.tile([1, 2 * HPC * HD], BF16, tag="bk")
            nc.gpsimd.dma_start(out=bkt[:], in_=bk_d[:])
            bvt = const.tile([1, HPC * HD], BF16, tag="bv")
            nc.gpsimd.dma_start(out=bvt[:], in_=bv_d[:])

        # ---- persistent per-head tiles ----
        qT = [qkpool.tile([128, S], BF16, tag=f"qT{h}", name=f"qT{h}")
              for h in range(HPC)]
        kT = [qkpool.tile([128, S], BF16, tag=f"kT{h}", name=f"kT{h}")
              for h in range(HPC)]
        va = [vpool.tile([128, HPC * HD], BF16, tag=f"va{c}", name=f"va{c}")
              for c in range(NKC)]
        stgT = [spool.tile([128, S], BF16, tag=f"sT{p}", name=f"sT{p}")
                for p in range(HPC // 2)]

        kdone = [[False] * NJ for _ in range(HPC)]
        qdone = [[False] * NJ for _ in range(HPC)]

        def emit_qkproj_group(which, h, g, prio=1500, use_act=False, halves=False):
            # one j-group of the q or k projection for head h: psum [128, 512]
            # (partitions = q1|q2 of the head), evicted bf16 into qT/kT.
            wt, bt, dst, done = (
                (wqt, bqt if with_bias else None, qT[h], qdone)
                if which == "q"
                else (wkt, bkt if with_bias else None, kT[h], kdone)
            )
            if done[h][g]:
                return
            done[h][g] = True
            # h0's k-groups emit as two 256-col half-groups: each half only
            # depends on half an hs j-slice DMA, so it runs as soon as that
            # half lands (keeps the very first block's exp cadence fed)
            nhalves = 2 if (halves and h == 0) else 1
            w = QB // nhalves
            # medium-high priority: the eviction feeds upcoming score chunks,
            # so it must beat combine/out-proj work on PE and DVE
            with tc.high_priority(offset=prio):
                for hv in range(nhalves):
                    cols = slice(g * QB + hv * w, g * QB + (hv + 1) * w)
                    ps = ps_mm.tile([128, 512], F32, tag="mm", name="psqk")
                    for c in range(NDI):
                        nc.tensor.matmul(
                            ps[:, 0:w],
                            lhsT=wt[:, c, h * 128:(h + 1) * 128],
                            rhs=hst[:, c, cols],
                            start=(c == 0),
                            stop=(with_bias is False and c == NDI - 1),
                        )
                    if with_bias:
                        nc.tensor.matmul(
                            ps[:, 0:w],
                            lhsT=bt[0:1, h * 128:(h + 1) * 128],
                            rhs=ones[0:1, 0:w],
                            start=False,
                            stop=True,
                        )
                    if use_act:
                        nc.scalar.copy(dst[:, cols], ps[:, 0:w])
                    else:
                        nc.vector.tensor_copy(dst[:, cols], ps[:, 0:w])

        def emit_vproj_chunk(sc):
            # v for s-chunk sc: psum [128 tokens, 256], evicted into va[sc].
            ps = ps_mm.tile([128, 512], F32, tag="mm", name="psv")
            for c in range(NDI):
                nc.tensor.matmul(
                    ps[:, 0:256],
                    lhsT=hst[:, c, sc * KC:(sc + 1) * KC],
                    rhs=wvt[:, c, :],
                    start=(c == 0),
                    stop=(with_bias is False and c == NDI - 1),
                )
            if with_bias:
                nc.tensor.matmul(
                    ps[:, 0:256],
                    lhsT=ones[0:1, 0:128],
                    rhs=bvt[0:1, :],
                    start=False,
                    stop=True,
                )
            nc.vector.tensor_copy(va[sc][:], ps[:, 0:256])

        def emit_score_exp(h, j, c):
            # transposed scores sT[k, q] for k-chunk c, both softmax halves,
            # then exp on ACT -> bf16 et tile [128, 2*QB].
            # top priority: every score chunk gates an ACT exp, and ACT is the
            # bottleneck engine — scores must never lose PE to filler work
            with tc.high_priority(offset=4000):
                sp = ps_sc.tile([128, 2 * QB], F32, tag="sp", name="sp")
                for half in range(2):
                    nc.tensor.matmul(
                        sp[:, half * QB:(half + 1) * QB],
                        lhsT=kT[h][half * 64:(half + 1) * 64, c * KC:(c + 1) * KC],
                        rhs=qT[h][half * 64:(half + 1) * 64, j * QB:(j + 1) * QB],
                        start=True,
                        stop=True,
                    )
                et = epool.tile([128, 2 * QB], BF16, tag="et", name="et")
                nc.scalar.activation(
                    et[:],
                    sp[:],
                    mybir.ActivationFunctionType.Exp,
                    bias=maskt[:, c:c + 1],
                    scale=float(HD) ** -0.5,
                )
            return et

        def emit_pv_chunk(h, c, et, pvt, smt):
            # PV + softmax-sum accumulation for one k-chunk: 8 64-row matmuls
            # into the packed pv bank, 8 1-row ones-matmuls into the sums
            # bank.  Only the first matmul of c==0 uses start=True per bank
            # (2KB zero-region covers the rest of the round).
            first, last = c == 0, c == NKC - 1
            for qc in range(NQC):
                for half in range(2):
                    sl = et[:, half * QB + qc * 128:half * QB + (qc + 1) * 128]
                    r = 2 * qc + half
                    nc.tensor.matmul(
                        pvt[:, r * HD:(r + 1) * HD],
                        lhsT=sl,
                        rhs=va[c][:, h * HD:(h + 1) * HD],
                        start=(first and r == 0),
                        stop=last,
                        skip_group_check=True,
                    )
                    nc.tensor.matmul(
                        smt[:, r:r + 1],
                        lhsT=sl,
                        rhs=ones[:, 0:1],
                        start=(first and r == 0),
                        stop=last,
                        skip_group_check=True,
                    )

        def emit_combine(h, j, pvt, smt, last=False):
            # stg[q, d] = pv1/sum1 - lam * pv2/sum2 per q-subchunk, then PE
            # transpose (plain matmul vs identity) into [d, q] stacked by
            # head parity, one DVE eviction per (h, j) into stgT.  The very
            # last block's t2 ops go on ACT (idle once the exps are done),
            # halving the serial DVE chain in the tail.
            rt = gpool.tile([128, 8], F32, tag="rt", name="rt")
            nc.vector.reciprocal(out=rt[:], in_=smt[:, 0:8])
            if last:
                rtl = gpool.tile([128, NQC], F32, tag="rtl", name="rtl")
                nc.vector.tensor_scalar(
                    out=rtl[:],
                    in0=rt[:, 0:8].rearrange("p (a b) -> p a b", b=2)[:, :, 1],
                    scalar1=float(lam),
                    scalar2=None,
                    op0=mult,
                )
            trp = ps_mm.tile([128, 512], F32, tag="mm", name="trp")
            hh = (h % 2) * 64
            # all t2 ops first, then the stg ops: halves the serial DVE chain
            t2s = []
            for qc in range(NQC):
                t2 = gpool.tile([128, HD], F32, tag="t2", name="t2")
                if last:
                    nc.scalar.activation(
                        t2[:],
                        pvt[:, (2 * qc + 1) * HD:(2 * qc + 2) * HD],
                        mybir.ActivationFunctionType.Copy,
                        scale=rtl[:, qc:qc + 1],
                    )
                else:
                    nc.vector.tensor_scalar(
                        out=t2[:],
                        in0=pvt[:, (2 * qc + 1) * HD:(2 * qc + 2) * HD],
                        scalar1=rt[:, 2 * qc + 1:2 * qc + 2],
                        scalar2=float(lam),
                        op0=mult,
                        op1=mult,
                    )
                t2s.append(t2)
            for qc in range(NQC):
                stg = gpool.tile([128, HD], BF16, tag="stg", name="stg")
                nc.vector.scalar_tensor_tensor(
                    out=stg[:],
                    in0=pvt[:, 2 * qc * HD:(2 * qc + 1) * HD],
                    scalar=rt[:, 2 * qc:2 * qc + 1],
                    in1=t2s[qc][:],
                    op0=mult,
                    op1=subtract,
                )
                nc.tensor.matmul(
                    trp[hh:hh + 64, qc * 128:(qc + 1) * 128],
                    lhsT=stg[:],
                    rhs=ident[:],
                    start=(qc == 0),
                    stop=True,
                    skip_group_check=True,
                )
            nc.vector.tensor_copy(
                stgT[h // 2][hh:hh + 64, j * QB:(j + 1) * QB],
                trp[hh:hh + 64, :],
            )

        def emit_outproj_chunk(j, dd, p, use_act=False):
            # per-head-pair partial out-projection for a PAIR of dout chunks
            # (dd = 0..3 -> douts 2dd, 2dd+1): two K=128 matmuls, two
            # evictions into one [128, 1024] tile, ONE output DMA (SP DMA
            # triggers cost 565ns of sequencer each — batching halves them).
            # The host sums the two pair partials per core.  The very last
            # block's evictions alternate onto ACT (idle once exps are done).
            ot = opool.tile([128, 1024], F16, tag="ot", name="ot")
            for i in range(2):
                d = 2 * dd + i
                ps = ps_mm.tile([128, 512], F32, tag="mm", name="pso")
                nc.tensor.matmul(
                    ps[:],
                    lhsT=wot[p][:, d * 128:(d + 1) * 128],
                    rhs=stgT[p][:, j * QB:(j + 1) * QB],
                    start=True,
                    stop=True,
                )
                if use_act and i % 2 == 0:
                    nc.scalar.copy(ot[:, i * 512:(i + 1) * 512], ps[:])
                else:
                    nc.vector.tensor_copy(ot[:, i * 512:(i + 1) * 512], ps[:])
            dst = out_d[p][2 * dd * 128:(2 * dd + 2) * 128,
                           j * QB:(j + 1) * QB].rearrange("(d p) s -> p d s", p=128)
            nc.sync.dma_start(
                out=dst, in_=ot[:].rearrange("p (d s) -> p d s", s=QB)
            )

        # ---- emission schedule ----
        # Primary stream: score chunks (they feed ACT, the bottleneck).
        # Fillers drain between chunks: v-projection first, then deferred PV
        # rounds (per-chunk granularity, rounds kept contiguous so the packed
        # pv bank only ever holds one accumulation round), next head's
        # projections (usually JIT-emitted and no-op'd here), out-proj.
        for _rep in range(repeat):
            fillers = deque()
            state = {"vp": 0, "pv": 0}
            for sc in range(NKC):
                fillers.append(("vp", lambda sc=sc: emit_vproj_chunk(sc)))

            def drain(n):
                for _ in range(n):
                    if not fillers:
                        return
                    kind, thunk = fillers.popleft()
                    thunk()
                    if kind in state:
                        state[kind] += 1 if kind == "vp" else -1

            backlog = []  # (h, j, [et tiles]) with PV not yet emitted

            def release_block(bh, bj, bets):
                holder = {}

                def pvchunk(c):
                    if not holder:
                        holder["pv"] = ps_pv.tile(
                            [128, 8 * HD], F32, tag="pv", name="pvt"
                        )
                        holder["sm"] = ps_sm.tile([128, 16], F32, tag="sm", name="smt")
                    emit_pv_chunk(bh, c, bets[c], holder["pv"], holder["sm"])

                for c in range(NKC):
                    fillers.append(("pv", lambda c=c: pvchunk(c)))
                fillers.append(
                    ("pv", lambda: emit_combine(bh, bj, holder["pv"], holder["sm"]))
                )
                state["pv"] += NKC + 1

            # ACT is idle until the first exp — its queue takes the first two
            # evictions so the DVE isn't a serial bottleneck at startup
            emit_qkproj_group("q", 0, 0, use_act=True)
            emit_qkproj_group("k", 0, 0, use_act=True)
            chunk_no = 0

            for h in range(HPC):
                for j in range(NJ):
                    emit_qkproj_group("q", h, j)
                    # prefetch upcoming q-projections near the front of the
                    # filler queue so their evictions land before those
                    # blocks' first scores (kills the block-boundary ACT gap)
                    bi = h * NJ + j
                    if bi:
                        nh, njx = divmod(bi + 2, NJ)
                        if nh < HPC:
                            fillers.appendleft(
                                ("qp", lambda a=nh, b=njx:
                                 emit_qkproj_group("q", a, b))
                            )
                    # next head's k-projection groups ahead of its first block
                    if j == 2 and h + 1 < HPC:
                        for g in reversed(range(NJ)):
                            fillers.appendleft(
                                ("kp", lambda a=h + 1, b=g:
                                 emit_qkproj_group("k", a, b))
                            )
                    # inline PV only once v-proj is done and no deferred round
                    # is still queued (the packed pv bank admits one round at
                    # a time; deferred rounds drain with priority below)
                    inline = state["vp"] == NKC and not backlog
                    holder = {}

                    def own_pv(c, h=h):
                        if not holder:
                            holder["pv"] = ps_pv.tile(
                                [128, 8 * HD], F32, tag="pv", name="pvt"
                            )
                            holder["sm"] = ps_sm.tile(
                                [128, 16], F32, tag="sm", name="smt"
                            )
                        emit_pv_chunk(h, c, ets[c], holder["pv"], holder["sm"])

                    ets = []
                    pend = deque()
                    for c in range(NKC):
                        # current block's k-groups outrank next-block prefetch;
                        # 2-chunk lead so the eviction lands before its scores
                        emit_qkproj_group(
                            "k", h, min(NJ - 1, (c + 2) // NQC),
                            prio=2500, halves=True,
                        )
                        # j0's q-prefetches wait until its k-groups are all
                        # emitted: the mm pool round-robin binds groups in
                        # emission order, so anything emitted between k-groups
                        # whose inputs land late would stall them all
                        if bi == 0 and c in (12, 14):
                            njx = c // 2 - 5  # 12 -> j1, 14 -> j2
                            fillers.appendleft(
                                ("qp", lambda b=njx: emit_qkproj_group("q", 0, b))
                            )
                        ets.append(emit_score_exp(h, j, c))
                        chunk_no += 1
                        if inline:
                            pend.append(c)
                        if state["pv"]:
                            # finish the deferred round first (bank exclusive)
                            drain(2)
                        elif inline and len(pend) > 2:
                            # lag-2 pipeline: PV trails the exp by two chunks
                            while len(pend) > 2:
                                own_pv(pend.popleft())
                            drain(1)
                        elif fillers and fillers[0][0] == "vp" and chunk_no <= NKC:
                            # j0 never drains vp: wv arrives after the hs
                            # slices, and a vp tile in the mm rotation would
                            # block the k-groups behind it until wv lands
                            pass
                        else:
                            drain(1)
                    if inline:
                        # the packed pv bank admits one round at a time: any
                        # deferred round must fully emit (incl. its combine)
                        # before this block's round opens
                        while state["pv"]:
                            drain(1)
                        while pend:
                            own_pv(pend.popleft())
                        emit_combine(h, j, holder["pv"], holder["sm"])
                    else:
                        backlog.append((h, j, ets))
                        if state["vp"] == NKC:
                            for blk in backlog:
                                release_block(*blk)
                            backlog = []
                    # pair-p out-projection for block j once its second head
                    # finished the block (h==1 -> pair 0, h==3 -> pair 1)
                    if h % 2 == 1:
                        last = h == HPC - 1 and j == NJ - 1
                        for dd in range(NDI // 2):
                            fillers.append(
                                ("op", lambda a=j, b=dd, c2=h // 2, ua=last:
                                 emit_outproj_chunk(a, b, c2, use_act=ua))
                            )
            drain(len(fillers))

    nc.compile()
    return nc


def _prep_inputs(hidden_states, attention_mask, Wq, bq, Wk, bk, Wv, bv, Wo):
    """Build the 8 per-core input maps (host-side shard + transpose + cast)."""
    in_maps = []
    hsT = [np.ascontiguousarray(hidden_states[b].T).astype(npbf16) for b in range(B)]
    maskc = [
        np.ascontiguousarray(
            ((1.0 - attention_mask[b]) * -10000.0).astype(np.float32).reshape(NKC, KC).T
        )
        for b in range(B)
    ]
    ident = np.eye(128, dtype=npbf16)
    for core in range(NCORES):
        b = core // (NCORES // B)
        hb = (core % (NCORES // B)) * HPC
        heads = range(hb, hb + HPC)
        qk_idx = np.concatenate(
            [np.r_[h * HD:(h + 1) * HD, D + h * HD:D + (h + 1) * HD] for h in heads]
        )
        v_idx = np.r_[hb * HD:(hb + HPC) * HD]
        in_maps.append(
            {
                "hst": hsT[b],
                "wq": np.ascontiguousarray(Wq[:, qk_idx]).astype(npbf16),
                "wk": np.ascontiguousarray(Wk[:, qk_idx]).astype(npbf16),
                "wv": np.ascontiguousarray(Wv[:, v_idx]).astype(npbf16),
                "wo": np.ascontiguousarray(Wo[v_idx, :]).astype(npbf16),
                "bq": bq[qk_idx].reshape(1, -1).astype(npbf16),
                "bk": bk[qk_idx].reshape(1, -1).astype(npbf16),
                "bv": bv[v_idx].reshape(1, -1).astype(npbf16),
                "maskc": maskc[b],
                "ident": ident,
            }
        )
    return in_maps


def kernel(
    hidden_states,
    attention_mask,
    Wq,
    bq,
    Wk,
    bk,
    Wv,
    bv,
    Wo,
    bo,
    lq1,
    lk1,
    lq2,
    lk2,
):
    global LAST_RESULTS
    args = [hidden_states, attention_mask, Wq, bq, Wk, bk, Wv, bv, Wo, bo]
    hidden_states, attention_mask, Wq, bq, Wk, bk, Wv, bv, Wo, bo = (
        np.asarray(a, dtype=np.float32) for a in args
    )
    lq1, lk1, lq2, lk2 = (np.asarray(a, dtype=np.float64) for a in (lq1, lk1, lq2, lk2))
    lam = float(np.exp(lq1 @ lk1) - np.exp(lq2 @ lk2) + 0.8)

    with_bias = not (
        np.all(bq == 0) and np.all(bk == 0) and np.all(bv == 0)
    )
    key = (round(lam, 9), with_bias)
    if key not in _BUILD_CACHE:
        _BUILD_CACHE.clear()
        _BUILD_CACHE[key] = _build(lam, with_bias)
    nc = _BUILD_CACHE[key]

    in_maps = _prep_inputs(hidden_states, attention_mask, Wq, bq, Wk, bk, Wv, bv, Wo)
    res = run_bass_kernel_spmd(nc, in_maps, core_ids=list(range(NCORES)), trace=TRACE)
    LAST_RESULTS = res

    out = np.empty((B, S, D), dtype=np.float32)
    gpb = NCORES // B
    for b in range(B):
        acc = res.results[b * gpb]["outT0"].astype(np.float32)
        acc += res.results[b * gpb]["outT1"]
        for g in range(1, gpb):
            acc += res.results[b * gpb + g]["outT0"]
            acc += res.results[b * gpb + g]["outT1"]
        out[b] = acc.T + bo[None, :]
    return out


# revision 68
# speedup vs baseline: 1.3332x; 1.0056x over previous
"""Differential attention kernel for Trainium2, 8-core SPMD.

Problem: B=2, S=2048, D=1024, 16 heads x 64 head-dim differential attention
(two softmaxes combined with a scalar lambda), with input/output projections.

Sharding: data-parallel over batch (2 groups of 4 cores) x tensor-parallel
over heads (4 heads per core). Each core computes q/k/v projections for its
4 heads, both attention softmaxes, and a partial output projection
(its heads' rows of Wo). Host sums the 4 partial outputs per batch, adds bo.

Design (driven by the TimelineSim cost model, where a matmul costs
out-free-size x pe_cycle and an ACT op costs free-size x act_cycle):
  - Projections produce transposed q/k ([dh 128 = q1|q2 stacked, S]) and
    v ([s, 4*64]) with K=128/M=128 matmuls (row-optimal).
  - Scores are computed transposed, sT[k, q], per 128-token k-chunk:
    two K=64 matmuls (halves on partition ranges 0:64 / 64:128).
  - exp() runs on ACT straight out of PSUM ([128, 1024] per chunk,
    double-buffered PSUM), mask folded into the per-partition bias and the
    1/sqrt(hd) scale into the activation scale.  ACT is the kernel's
    bottleneck engine (~266 us busy), so nothing else runs on ACT.
  - PV is orientation-flipped vs the baseline: out[q, d] with lhsT = et
    chunk [k 128, q 128] and rhs = v [k 128, d 64] -> 64-row matmuls at
    full K=128/M=128 (half the PE rows of the [d, q] orientation).  All
    8 accumulators (4 q-subchunks x 2 softmax halves) pack into ONE psum
    bank; softmax sums accumulate via 1-row ones-matmuls into a second
    bank.  PSUM 2KB zero-region semantics: only the first matmul touching
    a bank per round uses start=True, the other regions' first writes
    clear their pending-zero bytes (fresh write), later writes accumulate.
  - Normalization needs no partition broadcast in this orientation:
    DVE reciprocal of the sums column + per-partition tensor_scalar ops
    combine the halves with lambda folded in.
  - stg [q, d] is PE-transposed (plain bf16 matmul against an identity)
    into [d, q] with head pairs stacked on partitions, so the output
    projection contracts K=128 (half the PE rows of per-head K=64).
All matmuls run in bf16 with fp32 PSUM accumulation; output partials ship
as fp16 and are reduced across cores in fp32 on the host.

Engine budget per core (cost model): ACT 266us (256 exp ops), PE ~255us
(~608k matmul rows), DVE ~90us.  The emission order keeps ACT dense:
score chunks are the primary stream; projection groups are emitted
just-in-time ahead of their consumers; PV work for early (h, j) blocks is
deferred (et tiles are held in SBUF) until the v-projection has drained,
then drains as filler between score chunks.
"""

import sys

sys.path.insert(0, "/opt/trn_rl_repo")

from collections import deque
from contextlib import ExitStack

import ml_dtypes
import numpy as np

import concourse.bacc as bacc
import concourse.tile as tile
from concourse import mybir
from concourse.bass_utils import run_bass_kernel_spmd

B, S, D = 2, 2048, 1024
NH, HD = 16, 64
NCORES = 8
HPC = 4              # heads per core
QB = 512             # q block (free dim of score matmuls)
NJ = S // QB         # 4
KC = 128             # k chunk (partition dim of transposed scores)
NKC = S // KC        # 16
NDI = D // 128       # 8 contraction chunks for projections
NQC = QB // 128      # 4 q-subchunks per block (PV output partition tiles)

BF16 = mybir.dt.bfloat16
F32 = mybir.dt.float32
F16 = mybir.dt.float16
npbf16 = ml_dtypes.bfloat16

_BUILD_CACHE = {}
TRACE = False
LAST_RESULTS = None


def _build(lam: float, with_bias: bool = True, repeat: int = 1):
    nc = bacc.Bacc(None, target_bir_lowering=False)
    mult = mybir.AluOpType.mult
    subtract = mybir.AluOpType.subtract

    hst_d = nc.dram_tensor("hst", [D, S], BF16, kind="ExternalInput")
    wq_d = nc.dram_tensor("wq", [D, 2 * HPC * HD], BF16, kind="ExternalInput")
    wk_d = nc.dram_tensor("wk", [D, 2 * HPC * HD], BF16, kind="ExternalInput")
    wv_d = nc.dram_tensor("wv", [D, HPC * HD], BF16, kind="ExternalInput")
    wo_d = nc.dram_tensor("wo", [HPC * HD, D], BF16, kind="ExternalInput")
    bq_d = nc.dram_tensor("bq", [1, 2 * HPC * HD], BF16, kind="ExternalInput")
    bk_d = nc.dram_tensor("bk", [1, 2 * HPC * HD], BF16, kind="ExternalInput")
    bv_d = nc.dram_tensor("bv", [1, HPC * HD], BF16, kind="ExternalInput")
    mask_d = nc.dram_tensor("maskc", [KC, NKC], F32, kind="ExternalInput")
    id_d = nc.dram_tensor("ident", [128, 128], BF16, kind="ExternalInput")
    out_d = [
        nc.dram_tensor(f"outT{p}", [D, S], F16, kind="ExternalOutput")
        for p in range(HPC // 2)
    ]

    with tile.TileContext(nc) as tc, ExitStack() as ctx:
        const = ctx.enter_context(tc.tile_pool(name="const", bufs=1))
        wpool = ctx.enter_context(tc.tile_pool(name="wpool", bufs=1))
        hpool = ctx.enter_context(tc.tile_pool(name="hpool", bufs=1))
        qkpool = ctx.enter_context(tc.tile_pool(name="qkpool", bufs=1))
        vpool = ctx.enter_context(tc.tile_pool(name="vpool", bufs=1))
        epool = ctx.enter_context(tc.tile_pool(name="epool", bufs=42))
        spool = ctx.enter_context(tc.tile_pool(name="spool", bufs=1))
        gpool = ctx.enter_context(tc.tile_pool(name="gpool", bufs=8))
        opool = ctx.enter_context(tc.tile_pool(name="opool", bufs=6))
        ps_mm = ctx.enter_context(tc.tile_pool(name="ps_mm", bufs=2, space="PSUM"))
        ps_sc = ctx.enter_context(tc.tile_pool(name="ps_sc", bufs=2, space="PSUM"))
        ps_pv = ctx.enter_context(tc.tile_pool(name="ps_pv", bufs=1, space="PSUM"))
        ps_sm = ctx.enter_context(tc.tile_pool(name="ps_sm", bufs=1, space="PSUM"))

        # ---- input DMAs.  The cost model serializes all DMA transfers on one
        # device (~324 GB/s), so order them by first use: head0's wk/wq
        # slices + hs j0 (unblocks the first score chunks ~6.5us in), then
        # mask, the remaining hs j-blocks (kproj JIT at chunks 4/8/12), wv,
        # the other heads' wk/wq, identity, wo.  One queue (Pool SWDGE) keeps
        # the global order deterministic and off the ACT/DVE engines. ----
        wkt = wpool.tile([128, NDI, 512], BF16, tag="wk", name="wkt")
        wqt = wpool.tile([128, NDI, 512], BF16, tag="wq", name="wqt")
        wvt = wpool.tile([128, NDI, 256], BF16, tag="wv", name="wvt")
        hst = hpool.tile([128, NDI, S], BF16, tag="hs", name="hst")
        wk_r = wk_d[:].rearrange("(c p) m -> p c m", p=128)
        wq_r = wq_d[:].rearrange("(c p) m -> p c m", p=128)
        wv_r = wv_d[:].rearrange("(c p) m -> p c m", p=128)
        hs_r = hst_d[:].rearrange("(c p) s -> p c s", p=128)
        maskt = const.tile([KC, NKC], F32, tag="mask")
        ident = const.tile([128, 128], BF16, tag="ident")
        ones = const.tile([128, QB], BF16, tag="ones")
        nc.gpsimd.memset(ones[:], 1.0)
        wot = [wpool.tile([128, D], BF16, tag=f"wo{p}", name=f"wo{p}")
               for p in range(HPC // 2)]

        nc.sync.dma_start(out=wqt[:, :, 0:128], in_=wq_r[:, :, 0:128])
        nc.scalar.dma_start(out=hst[:, 0:4, 0:QB], in_=hs_r[:, 0:4, 0:QB])
        nc.sync.dma_start(out=hst[:, 4:8, 0:QB], in_=hs_r[:, 4:8, 0:QB])
        nc.sync.dma_start(out=wkt[:, :, 0:128], in_=wk_r[:, :, 0:128])
        nc.scalar.dma_start(out=maskt[:], in_=mask_d[:])
        nc.scalar.dma_start(out=wvt[:], in_=wv_r[:])
        for jh in range(2, 2 * NJ):  # j1..j3 in 256-column halves
            nc.sync.dma_start(
                out=hst[:, :, jh * 256:(jh + 1) * 256],
                in_=hs_r[:, :, jh * 256:(jh + 1) * 256],
            )
        nc.sync.dma_start(out=ident[:], in_=id_d[:])
        nc.sync.dma_start(out=wkt[:, :, 128:512], in_=wk_r[:, :, 128:512])
        nc.sync.dma_start(out=wqt[:, :, 128:512], in_=wq_r[:, :, 128:512])
        for p in range(HPC // 2):
            nc.sync.dma_start(out=wot[p][:], in_=wo_d[p * 128:(p + 1) * 128, :])

        # PE p-state warm-up: ~3us of throwaway matmuls while the first DMAs
        # land, so the projection/score chain starts at full clock
        wps = ps_mm.tile([128, 512], F32, tag="mm", name="wps")
        for i in range(7):
            nc.tensor.matmul(
                wps[:], lhsT=ones[:, 0:128], rhs=ones[:],
                start=(i == 0), stop=(i == 6),
            )
        if with_bias:
            bqt = const.tile([1, 2 * HPC * HD], BF16, tag="bq")
            nc.gpsimd.dma_start(out=bqt[:], in_=bq_d[:])
            bkt = const.tile([1, 2 * HPC * HD], BF16, tag="bk")
            nc.gpsimd.dma_start(out=bkt[:], in_=bk_d[:])
            bvt = const.tile([1, HPC * HD], BF16, tag="bv")
            nc.gpsimd.dma_start(out=bvt[:], in_=bv_d[:])

        # ---- persistent per-head tiles ----
        qT = [qkpool.tile([128, S], BF16, tag=f"qT{h}", name=f"qT{h}")
              for h in range(HPC)]
        kT = [qkpool.tile([128, S], BF16, tag=f"kT{h}", name=f"kT{h}")
              for h in range(HPC)]
        va = [vpool.tile([128, HPC * HD], BF16, tag=f"va{c}", name=f"va{c}")
              for c in range(NKC)]
        stgT = [spool.tile([128, S], BF16, tag=f"sT{p}", name=f"sT{p}")
                for p in range(HPC // 2)]

        kdone = [[False] * NJ for _ in range(HPC)]
        qdone = [[False] * NJ for _ in range(HPC)]

        def emit_qkproj_group(which, h, g, prio=1500, use_act=False, halves=False):
            # one j-group of the q or k projection for head h: psum [128, 512]
            # (partitions = q1|q2 of the head), evicted bf16 into qT/kT.
            wt, bt, dst, done = (
                (wqt, bqt if with_bias else None, qT[h], qdone)
                if which == "q"
                else (wkt, bkt if with_bias else None, kT[h], kdone)
            )
            if done[h][g]:
                return
            done[h][g] = True
            # h0's k-groups emit as two 256-col half-groups: each half only
            # depends on half an hs j-slice DMA, so it runs as soon as that
            # half lands (keeps the very first block's exp cadence fed)
            nhalves = 2 if (halves and h == 0) else 1
            w = QB // nhalves
            # medium-high priority: the eviction feeds upcoming score chunks,
            # so it must beat combine/out-proj work on PE and DVE
            with tc.high_priority(offset=prio):
                for hv in range(nhalves):
                    cols = slice(g * QB + hv * w, g * QB + (hv + 1) * w)
                    ps = ps_mm.tile([128, 512], F32, tag="mm", name="psqk")
                    for c in range(NDI):
                        nc.tensor.matmul(
                            ps[:, 0:w],
                            lhsT=wt[:, c, h * 128:(h + 1) * 128],
                            rhs=hst[:, c, cols],
                            start=(c == 0),
                            stop=(with_bias is False and c == NDI - 1),
                        )
                    if with_bias:
                        nc.tensor.matmul(
                            ps[:, 0:w],
                            lhsT=bt[0:1, h * 128:(h + 1) * 128],
                            rhs=ones[0:1, 0:w],
                            start=False,
                            stop=True,
                        )
                    if use_act and hv == 0:
                        nc.scalar.copy(dst[:, cols], ps[:, 0:w])
                    else:
                        nc.vector.tensor_copy(dst[:, cols], ps[:, 0:w])

        def emit_vproj_chunk(sc):
            # v for s-chunk sc: psum [128 tokens, 256], evicted into va[sc].
            ps = ps_mm.tile([128, 512], F32, tag="mm", name="psv")
            for c in range(NDI):
                nc.tensor.matmul(
                    ps[:, 0:256],
                    lhsT=hst[:, c, sc * KC:(sc + 1) * KC],
                    rhs=wvt[:, c, :],
                    start=(c == 0),
                    stop=(with_bias is False and c == NDI - 1),
                )
            if with_bias:
                nc.tensor.matmul(
                    ps[:, 0:256],
                    lhsT=ones[0:1, 0:128],
                    rhs=bvt[0:1, :],
                    start=False,
                    stop=True,
                )
            nc.vector.tensor_copy(va[sc][:], ps[:, 0:256])

        def emit_score_exp(h, j, c):
            # transposed scores sT[k, q] for k-chunk c, both softmax halves,
            # then exp on ACT -> bf16 et tile [128, 2*QB].
            # top priority: every score chunk gates an ACT exp, and ACT is the
            # bottleneck engine — scores must never lose PE to filler work
            with tc.high_priority(offset=4000):
                sp = ps_sc.tile([128, 2 * QB], F32, tag="sp", name="sp")
                for half in range(2):
                    nc.tensor.matmul(
                        sp[:, half * QB:(half + 1) * QB],
                        lhsT=kT[h][half * 64:(half + 1) * 64, c * KC:(c + 1) * KC],
                        rhs=qT[h][half * 64:(half + 1) * 64, j * QB:(j + 1) * QB],
                        start=True,
                        stop=True,
                    )
                et = epool.tile([128, 2 * QB], BF16, tag="et", name="et")
                nc.scalar.activation(
                    et[:],
                    sp[:],
                    mybir.ActivationFunctionType.Exp,
                    bias=maskt[:, c:c + 1],
                    scale=float(HD) ** -0.5,
                )
            return et

        def emit_pv_chunk(h, c, et, pvt, smt):
            # PV + softmax-sum accumulation for one k-chunk: 8 64-row matmuls
            # into the packed pv bank, 8 1-row ones-matmuls into the sums
            # bank.  Only the first matmul of c==0 uses start=True per bank
            # (2KB zero-region covers the rest of the round).
            first, last = c == 0, c == NKC - 1
            for qc in range(NQC):
                for half in range(2):
                    sl = et[:, half * QB + qc * 128:half * QB + (qc + 1) * 128]
                    r = 2 * qc + half
                    nc.tensor.matmul(
                        pvt[:, r * HD:(r + 1) * HD],
                        lhsT=sl,
                        rhs=va[c][:, h * HD:(h + 1) * HD],
                        start=(first and r == 0),
                        stop=last,
                        skip_group_check=True,
                    )
                    nc.tensor.matmul(
                        smt[:, r:r + 1],
                        lhsT=sl,
                        rhs=ones[:, 0:1],
                        start=(first and r == 0),
                        stop=last,
                        skip_group_check=True,
                    )

        def emit_combine(h, j, pvt, smt, last=False):
            # stg[q, d] = pv1/sum1 - lam * pv2/sum2 per q-subchunk, then PE
            # transpose (plain matmul vs identity) into [d, q] stacked by
            # head parity, one DVE eviction per (h, j) into stgT.  The very
            # last block's t2 ops go on ACT (idle once the exps are done),
            # halving the serial DVE chain in the tail.
            rt = gpool.tile([128, 8], F32, tag="rt", name="rt")
            nc.vector.reciprocal(out=rt[:], in_=smt[:, 0:8])
            if last:
                rtl = gpool.tile([128, NQC], F32, tag="rtl", name="rtl")
                nc.vector.tensor_scalar(
                    out=rtl[:],
                    in0=rt[:, 0:8].rearrange("p (a b) -> p a b", b=2)[:, :, 1],
                    scalar1=float(lam),
                    scalar2=None,
                    op0=mult,
                )
            trp = ps_mm.tile([128, 512], F32, tag="mm", name="trp")
            hh = (h % 2) * 64
            # all t2 ops first, then the stg ops: halves the serial DVE chain
            t2s = []
            for qc in range(NQC):
                t2 = gpool.tile([128, HD], F32, tag="t2", name="t2")
                if last:
                    nc.scalar.activation(
                        t2[:],
                        pvt[:, (2 * qc + 1) * HD:(2 * qc + 2) * HD],
                        mybir.ActivationFunctionType.Copy,
                        scale=rtl[:, qc:qc + 1],
                    )
                else:
                    nc.vector.tensor_scalar(
                        out=t2[:],
                        in0=pvt[:, (2 * qc + 1) * HD:(2 * qc + 2) * HD],
                        scalar1=rt[:, 2 * qc + 1:2 * qc + 2],
                        scalar2=float(lam),
                        op0=mult,
                        op1=mult,
                    )
                t2s.append(t2)
            for qc in range(NQC):
                stg = gpool.tile([128, HD], BF16, tag="stg", name="stg")
                nc.vector.scalar_tensor_tensor(
                    out=stg[:],
                    in0=pvt[:, 2 * qc * HD:(2 * qc + 1) * HD],
                    scalar=rt[:, 2 * qc:2 * qc + 1],
                    in1=t2s[qc][:],
                    op0=mult,
                    op1=subtract,
                )
                nc.tensor.matmul(
                    trp[hh:hh + 64, qc * 128:(qc + 1) * 128],
                    lhsT=stg[:],
                    rhs=ident[:],
                    start=(qc == 0),
                    stop=True,
                    skip_group_check=True,
                )
            nc.vector.tensor_copy(
                stgT[h // 2][hh:hh + 64, j * QB:(j + 1) * QB],
                trp[hh:hh + 64, :],
            )

        def emit_outproj_chunk(j, dd, p, use_act=False):
            # per-head-pair partial out-projection for a PAIR of dout chunks
            # (dd = 0..3 -> douts 2dd, 2dd+1): two K=128 matmuls, two
            # evictions into one [128, 1024] tile, ONE output DMA (SP DMA
            # triggers cost 565ns of sequencer each — batching halves them).
            # The host sums the two pair partials per core.  The very last
            # block's evictions alternate onto ACT (idle once exps are done).
            ot = opool.tile([128, 1024], F16, tag="ot", name="ot")
            for i in range(2):
                d = 2 * dd + i
                ps = ps_mm.tile([128, 512], F32, tag="mm", name="pso")
                nc.tensor.matmul(
                    ps[:],
                    lhsT=wot[p][:, d * 128:(d + 1) * 128],
                    rhs=stgT[p][:, j * QB:(j + 1) * QB],
                    start=True,
                    stop=True,
                )
                if use_act and i % 2 == 0:
                    nc.scalar.copy(ot[:, i * 512:(i + 1) * 512], ps[:])
                else:
                    nc.vector.tensor_copy(ot[:, i * 512:(i + 1) * 512], ps[:])
            dst = out_d[p][2 * dd * 128:(2 * dd + 2) * 128,
                           j * QB:(j + 1) * QB].rearrange("(d p) s -> p d s", p=128)
            nc.sync.dma_start(
                out=dst, in_=ot[:].rearrange("p (d s) -> p d s", s=QB)
            )

        # ---- emission schedule ----
        # Primary stream: score chunks (they feed ACT, the bottleneck).
        # Fillers drain between chunks: v-projection first, then deferred PV
        # rounds (per-chunk granularity, rounds kept contiguous so the packed
        # pv bank only ever holds one accumulation round), next head's
        # projections (usually JIT-emitted and no-op'd here), out-proj.
        for _rep in range(repeat):
            fillers = deque()
            state = {"vp": 0, "pv": 0}
            for sc in range(NKC):
                fillers.append(("vp", lambda sc=sc: emit_vproj_chunk(sc)))

            def drain(n):
                for _ in range(n):
                    if not fillers:
                        return
                    kind, thunk = fillers.popleft()
                    thunk()
                    if kind in state:
                        state[kind] += 1 if kind == "vp" else -1

            backlog = []  # (h, j, [et tiles]) with PV not yet emitted

            def release_block(bh, bj, bets):
                holder = {}

                def pvchunk(c):
                    if not holder:
                        holder["pv"] = ps_pv.tile(
                            [128, 8 * HD], F32, tag="pv", name="pvt"
                        )
                        holder["sm"] = ps_sm.tile([128, 16], F32, tag="sm", name="smt")
                    emit_pv_chunk(bh, c, bets[c], holder["pv"], holder["sm"])

                for c in range(NKC):
                    fillers.append(("pv", lambda c=c: pvchunk(c)))
                fillers.append(
                    ("pv", lambda: emit_combine(bh, bj, holder["pv"], holder["sm"]))
                )
                state["pv"] += NKC + 1

            # ACT is idle until the first exp — its queue takes the first two
            # evictions so the DVE isn't a serial bottleneck at startup
            emit_qkproj_group("q", 0, 0, use_act=True)
            emit_qkproj_group("k", 0, 0, use_act=True)
            chunk_no = 0
            pending_close = None

            for h in range(HPC):
                for j in range(NJ):
                    emit_qkproj_group("q", h, j)
                    # prefetch upcoming q-projections near the front of the
                    # filler queue so their evictions land before those
                    # blocks' first scores (kills the block-boundary ACT gap)
                    bi = h * NJ + j
                    if bi:
                        nh, njx = divmod(bi + 2, NJ)
                        if nh < HPC:
                            fillers.appendleft(
                                ("qp", lambda a=nh, b=njx:
                                 emit_qkproj_group("q", a, b))
                            )
                    # next head's k-projection groups ahead of its first block
                    if j == 2 and h + 1 < HPC:
                        for g in reversed(range(NJ)):
                            fillers.appendleft(
                                ("kp", lambda a=h + 1, b=g:
                                 emit_qkproj_group("k", a, b))
                            )
                    # inline PV only once v-proj is done and no deferred round
                    # is still queued (the packed pv bank admits one round at
                    # a time; deferred rounds drain with priority below)
                    inline = bi >= 2
                    holder = {}

                    def own_pv(c, h=h, holder=holder, ets=None):
                        if not holder:
                            holder["pv"] = ps_pv.tile(
                                [128, 8 * HD], F32, tag="pv", name="pvt"
                            )
                            holder["sm"] = ps_sm.tile(
                                [128, 16], F32, tag="sm", name="smt"
                            )
                        emit_pv_chunk(h, c, ets[c], holder["pv"], holder["sm"])

                    ets = []
                    pend = deque()
                    for c in range(NKC):
                        # current block's k-groups outrank next-block prefetch;
                        # 2-chunk lead so the eviction lands before its scores
                        emit_qkproj_group(
                            "k", h, min(NJ - 1, (c + 2) // NQC),
                            prio=2500, halves=True,
                        )
                        # j0's q-prefetches wait until its k-groups are all
                        # emitted: the mm pool round-robin binds groups in
                        # emission order, so anything emitted between k-groups
                        # whose inputs land late would stall them all
                        if bi == 0 and c in (12, 14):
                            njx = c // 2 - 5  # 12 -> j1, 14 -> j2
                            fillers.appendleft(
                                ("qp", lambda b=njx: emit_qkproj_group("q", 0, b))
                            )
                        ets.append(emit_score_exp(h, j, c))
                        chunk_no += 1
                        # the previous block's tail (pv flush + combine +
                        # out-proj queueing) emits AFTER this block's first
                        # two scores, so it never delays the exp cadence
                        if c == 1 and pending_close:
                            pending_close()
                            pending_close = None
                        if inline:
                            pend.append(c)
                        _ets = ets
                        if state["pv"]:
                            # finish the deferred round first (bank exclusive)
                            drain(2)
                        elif (inline and len(pend) > 2
                              and pending_close is None):
                            # lag-2 pipeline: PV trails the exp by two chunks
                            while len(pend) > 2:
                                own_pv(pend.popleft(), ets=_ets)
                            drain(1)
                        elif fillers and fillers[0][0] == "vp" and chunk_no <= 3:
                            # j0 never drains vp: wv arrives after the hs
                            # slices, and a vp tile in the mm rotation would
                            # block the k-groups behind it until wv lands
                            pass
                        else:
                            drain(1)

                    def close_block(inline=inline, pend=pend, holder=holder,
                                    ets=ets, h=h, j=j):
                        if inline:
                            # the packed pv bank admits one round at a time:
                            # any deferred round must fully emit (incl. its
                            # combine) before this block's round closes
                            while state["pv"]:
                                drain(1)
                            while pend:
                                own_pv(pend.popleft(), h, holder, ets)
                            emit_combine(h, j, holder["pv"], holder["sm"])
                        else:
                            backlog.append((h, j, ets))
                            if state["vp"] == NKC:
                                for blk in backlog:
                                    release_block(*blk)
                                backlog.clear()
                        # pair-p out-projection for block j once its second
                        # head finished it (h==1 -> pair 0, h==3 -> pair 1)
                        if h % 2 == 1:
                            last = h == HPC - 1 and j == NJ - 1
                            for dd in range(NDI // 2):
                                fillers.append(
                                    ("op", lambda a=j, b=dd, c2=h // 2,
                                     ua=last:
                                     emit_outproj_chunk(a, b, c2, use_act=ua))
                                )

                    pending_close = close_block
            if pending_close:
                pending_close()
            drain(len(fillers))

    nc.compile()
    return nc


def _prep_inputs(hidden_states, attention_mask, Wq, bq, Wk, bk, Wv, bv, Wo):
    """Build the 8 per-core input maps (host-side shard + transpose + cast)."""
    in_maps = []
    hsT = [np.ascontiguousarray(hidden_states[b].T).astype(npbf16) for b in range(B)]
    maskc = [
        np.ascontiguousarray(
            ((1.0 - attention_mask[b]) * -10000.0).astype(np.float32).reshape(NKC, KC).T
        )
        for b in range(B)
    ]
    ident = np.eye(128, dtype=npbf16)
    for core in range(NCORES):
        b = core // (NCORES // B)
        hb = (core % (NCORES // B)) * HPC
        heads = range(hb, hb + HPC)
        qk_idx = np.concatenate(
            [np.r_[h * HD:(h + 1) * HD, D + h * HD:D + (h + 1) * HD] for h in heads]
        )
        v_idx = np.r_[hb * HD:(hb + HPC) * HD]
        in_maps.append(
            {
                "hst": hsT[b],
                "wq": np.ascontiguousarray(Wq[:, qk_idx]).astype(npbf16),
                "wk": np.ascontiguousarray(Wk[:, qk_idx]).astype(npbf16),
                "wv": np.ascontiguousarray(Wv[:, v_idx]).astype(npbf16),
                "wo": np.ascontiguousarray(Wo[v_idx, :]).astype(npbf16),
                "bq": bq[qk_idx].reshape(1, -1).astype(npbf16),
                "bk": bk[qk_idx].reshape(1, -1).astype(npbf16),
                "bv": bv[v_idx].reshape(1, -1).astype(npbf16),
                "maskc": maskc[b],
                "ident": ident,
            }
        )
    return in_maps


def kernel(
    hidden_states,
    attention_mask,
    Wq,
    bq,
    Wk,
    bk,
    Wv,
    bv,
    Wo,
    bo,
    lq1,
    lk1,
    lq2,
    lk2,
):
    global LAST_RESULTS
    args = [hidden_states, attention_mask, Wq, bq, Wk, bk, Wv, bv, Wo, bo]
    hidden_states, attention_mask, Wq, bq, Wk, bk, Wv, bv, Wo, bo = (
        np.asarray(a, dtype=np.float32) for a in args
    )
    lq1, lk1, lq2, lk2 = (np.asarray(a, dtype=np.float64) for a in (lq1, lk1, lq2, lk2))
    lam = float(np.exp(lq1 @ lk1) - np.exp(lq2 @ lk2) + 0.8)

    with_bias = not (
        np.all(bq == 0) and np.all(bk == 0) and np.all(bv == 0)
    )
    key = (round(lam, 9), with_bias)
    if key not in _BUILD_CACHE:
        _BUILD_CACHE.clear()
        _BUILD_CACHE[key] = _build(lam, with_bias)
    nc = _BUILD_CACHE[key]

    in_maps = _prep_inputs(hidden_states, attention_mask, Wq, bq, Wk, bk, Wv, bv, Wo)
    res = run_bass_kernel_spmd(nc, in_maps, core_ids=list(range(NCORES)), trace=TRACE)
    LAST_RESULTS = res

    out = np.empty((B, S, D), dtype=np.float32)
    gpb = NCORES // B
    for b in range(B):
        acc = res.results[b * gpb]["outT0"].astype(np.float32)
        acc += res.results[b * gpb]["outT1"]
        for g in range(1, gpb):
            acc += res.results[b * gpb + g]["outT0"]
            acc += res.results[b * gpb + g]["outT1"]
        out[b] = acc.T + bo[None, :]
    return out
